# revision 10
# baseline (speedup 1.0000x reference)
"""Trainium2 Bass kernel for a GPT-style decoder block (B=2, T=2048, C=768, H=12).

Sharding v2: 8 cores = 2 batches x 4 head-groups. Core (b, g) runs attention
for heads {3g, 3g+1, 3g+2} over ALL T rows (full causal triangle), then the
cores of a batch exchange attention outputs with two bf16 ReduceScatters so
each core runs LN2+MLP on a disjoint block-cyclic quarter of the rows
(strips rows [512*ci + 128*g, +128) for ci in 0..3).

The SPMD program is identical on every core; all core-dependence lives in the
inputs: sliced QKV weights, plus one-hot select tensors (msel / selident) that
route data into the right ReduceScatter slot and select the core's own rows
out of replicated LN1 activations.

Attention layout: scoresT[j, i] = k_i . q_j (reference computes K @ Q^T), so
the moving dim of the score matmuls is i (own-output tokens) and softmax sums
over partitions j via a ones-column that rides in V. P@V runs in fp8e4 with
MatmulPerfMode.DoubleRow (2x PE throughput); everything else is bf16 with
fp32 accumulation. Causality: score matmuls only cover i >= j-block start;
the diagonal 128-block gets -1e30 added pre-exp; sub-diagonal exp slots are
memset to 0.

LN gamma/beta are folded into the adjacent matmul weights/biases host-side.
"""

import os

import numpy as np

B, T, C = 2, 2048, 768
H, DH = 12, 64
HL = 3               # heads per core
F = 4 * C
NT = T // 128        # 16 token tiles
NC = C // 128        # 6 channel chunks
NF = F // 128        # 24 hidden chunks
EPS = 1e-3
NEG = -1e30

_CACHE = {}


def _build_program():
    import concourse.bass as bass  # noqa: F401
    import concourse.mybir as mybir
    import concourse.tile as tile
    from concourse import bacc

    dt = mybir.dt
    f32 = dt.float32
    bf16 = dt.bfloat16
    f8 = dt.float8e4
    AF = mybir.ActivationFunctionType
    ALU = mybir.AluOpType
    DR = mybir.MatmulPerfMode.DoubleRow

    nc = bacc.Bacc("TRN2", target_bir_lowering=False, debug=False, num_devices=8)

    # ---- DRAM I/O ----
    x_d = nc.dram_tensor("x", [T, C], bf16, kind="ExternalInput")
    wq_d = nc.dram_tensor("wq", [C, HL * DH], bf16, kind="ExternalInput")
    wk_d = nc.dram_tensor("wk", [C, HL * DH], bf16, kind="ExternalInput")
    wv_d = nc.dram_tensor("wv", [C, HL * DH], bf16, kind="ExternalInput")
    bq_d = nc.dram_tensor("bq", [128, 2], f32, kind="ExternalInput")
    bk_d = nc.dram_tensor("bk", [128, 2], f32, kind="ExternalInput")
    bv_d = nc.dram_tensor("bv", [1, HL * DH], bf16, kind="ExternalInput")
    w1_d = nc.dram_tensor("w1", [C, F], bf16, kind="ExternalInput")
    b1_d = nc.dram_tensor("b1", [128, NF], f32, kind="ExternalInput")
    w2_d = nc.dram_tensor("w2", [F, C], bf16, kind="ExternalInput")
    b2_d = nc.dram_tensor("b2", [1, C], bf16, kind="ExternalInput")
    g1_d = nc.dram_tensor("g1", [1, C], bf16, kind="ExternalInput")
    b1r_d = nc.dram_tensor("b1r", [1, C], bf16, kind="ExternalInput")
    negtri_d = nc.dram_tensor("negtri", [128, 128], f32, kind="ExternalInput")
    ident_d = nc.dram_tensor("ident", [128, 128], bf16, kind="ExternalInput")
    msel_d = nc.dram_tensor("msel", [128, 4], f32, kind="ExternalInput")
    selid_d = nc.dram_tensor("selid", [128, 512], bf16, kind="ExternalInput")
    out_d = nc.dram_tensor("out", [512, C], f32, kind="ExternalOutput")

    with tile.TileContext(nc) as tc:
        with (
            tc.tile_pool(name="const", bufs=1) as constp,
            tc.tile_pool(name="dram", bufs=1, space="DRAM") as dramp,
            tc.tile_pool(name="psA", bufs=2, space="PSUM") as psA,
            tc.tile_pool(name="psPV", bufs=3, space="PSUM") as psPV,
            tc.tile_pool(name="psTr", bufs=2, space="PSUM") as psTr,
        ):
            # ---- constants ----
            negtri = constp.tile([128, 128], f32)
            nc.sync.dma_start(negtri[:], negtri_d[:])
            ident = constp.tile([128, 128], bf16)
            nc.sync.dma_start(ident[:], ident_d[:])
            msel = constp.tile([128, 4], f32)
            nc.sync.dma_start(msel[:], msel_d[:])
            selid = constp.tile([128, 512], bf16)
            nc.sync.dma_start(selid[:], selid_d[:])
            bqs = constp.tile([128, 2], f32)
            nc.sync.dma_start(bqs[:], bq_d[:])
            bks = constp.tile([128, 2], f32)
            nc.sync.dma_start(bks[:], bk_d[:])
            b1s = constp.tile([128, NF], f32)
            nc.sync.dma_start(b1s[:], b1_d[:])
            ones_col = constp.tile([1, 128], bf16)
            nc.vector.memset(ones_col[:], 1.0)
            eps_t = constp.tile([128, 1], f32)
            nc.vector.memset(eps_t[:], EPS)

            # broadcast ln1 gamma/beta rows to [128, C] via rank-1 matmuls
            g1s = constp.tile([1, C], bf16)
            nc.sync.dma_start(g1s[:], g1_d[:])
            b1rs = constp.tile([1, C], bf16)
            nc.sync.dma_start(b1rs[:], b1r_d[:])
            g1b = constp.tile([128, C], f32)
            b1rb = constp.tile([128, C], f32)
            for dst, src in ((g1b, g1s), (b1rb, b1rs)):
                for gg in range(2):
                    ps = psA.tile([128, 512], f32, tag="psA", name="psbc")
                    nc.tensor.matmul(
                        ps[:, 0:384], ones_col[:], src[:, gg * 384:(gg + 1) * 384],
                        start=True, stop=True,
                    )
                    nc.vector.tensor_copy(dst[:, gg * 384:(gg + 1) * 384], ps[:, 0:384])

            # persistent activations
            xn_bf = [constp.tile([128, C], bf16, name=f"xnbf{tb}") for tb in range(NT)]

            with (
                tc.tile_pool(name="QK", bufs=1) as qkp,
                tc.tile_pool(name="V8", bufs=1) as v8p,
            ):
                QT_ab = qkp.tile([128, T], bf16, name="QT_ab")
                QT_c = qkp.tile([64, T], bf16, name="QT_c")
                KT_ab = qkp.tile([128, T], bf16, name="KT_ab")
                KT_c = qkp.tile([64, T], bf16, name="KT_c")
                Vbf = [v8p.tile([128, HL, DH + 1], bf16, name=f"V_{tb}")
                       for tb in range(NT)]

                with tc.tile_pool(name="xnT", bufs=1) as xnTp:
                    xnT = [xnTp.tile([128, T], bf16, name=f"xnT{cb}") for cb in range(NC)]

                    # ===== Phase A: LN1 over all T + transpose to xnT =====
                    with (
                        tc.tile_pool(name="xin", bufs=3) as xinp,
                        tc.tile_pool(name="stat", bufs=4) as statp,
                    ):
                        for tb in range(NT):
                            xt = xinp.tile([128, C], bf16, tag="xt", name="xt")
                            nc.sync.dma_start(xt[:], x_d[tb * 128:(tb + 1) * 128, :])
                            st6 = statp.tile([128, 2, 6], f32, tag="st6", name="st6")
                            for gg in range(2):
                                nc.vector.bn_stats(
                                    st6[:, gg, :], xt[:, gg * 384:(gg + 1) * 384]
                                )
                            st2 = statp.tile([128, 2], f32, tag="st2", name="st2")
                            nc.vector.bn_aggr(st2[:], st6[:])
                            std = statp.tile([128, 1], f32, tag="std", name="std")
                            nc.scalar.activation(std[:], st2[:, 1:2], AF.Sqrt, bias=eps_t[:])
                            rstd = statp.tile([128, 1], f32, tag="rstd", name="rstd")
                            nc.vector.reciprocal(rstd[:], std[:])
                            nmb = statp.tile([128, 1], f32, tag="nmb", name="nmb")
                            nc.vector.tensor_scalar(
                                nmb[:], st2[:, 0:1], rstd[:], -1.0,
                                op0=ALU.mult, op1=ALU.mult,
                            )
                            nc.scalar.activation(
                                xn_bf[tb][:], xt[:], AF.Identity,
                                bias=nmb[:], scale=rstd[:],
                            )
                            for cb in range(NC):
                                tp = psTr.tile([128, 128], bf16, tag="psTr", name="tp")
                                nc.tensor.matmul(
                                    tp[:], xn_bf[tb][:, cb * 128:(cb + 1) * 128],
                                    ident[:], is_transpose=True, start=True, stop=True,
                                )
                                if cb % 2 == 0:
                                    nc.scalar.copy(
                                        xnT[cb][:, tb * 128:(tb + 1) * 128], tp[:])
                                else:
                                    nc.vector.tensor_copy(
                                        xnT[cb][:, tb * 128:(tb + 1) * 128], tp[:])

                    # ===== Phase B: QKV projections (3 heads) =====
                    with tc.tile_pool(name="wqkv", bufs=1) as wp:
                        wq = [wp.tile([128, HL * DH], bf16, name=f"wq{cb}") for cb in range(NC)]
                        wk = [wp.tile([128, HL * DH], bf16, name=f"wk{cb}") for cb in range(NC)]
                        wv = [wp.tile([128, HL * DH], bf16, name=f"wv{cb}") for cb in range(NC)]
                        for cb in range(NC):
                            nc.sync.dma_start(wq[cb][:], wq_d[cb * 128:(cb + 1) * 128, :])
                            nc.sync.dma_start(wk[cb][:], wk_d[cb * 128:(cb + 1) * 128, :])
                            nc.sync.dma_start(wv[cb][:], wv_d[cb * 128:(cb + 1) * 128, :])
                        bvs = wp.tile([1, HL * DH], bf16, name="bvs")
                        nc.sync.dma_start(bvs[:], bv_d[:])

                        for jb in range(T // 512):
                            sl = slice(jb * 512, (jb + 1) * 512)
                            for (W, dst_ab, dst_c, bias) in (
                                (wq, QT_ab, QT_c, bqs),
                                (wk, KT_ab, KT_c, bks),
                            ):
                                ps = psA.tile([128, 512], f32, tag="psA", name="psq")
                                for cb in range(NC):
                                    nc.tensor.matmul(
                                        ps[:], W[cb][:, 0:128], xnT[cb][:, sl],
                                        start=(cb == 0), stop=(cb == NC - 1),
                                    )
                                nc.vector.tensor_scalar_add(
                                    dst_ab[:, sl], ps[:], bias[:, 0:1])
                                ps2 = psA.tile([128, 512], f32, tag="psA", name="psq2")
                                for cb in range(NC):
                                    nc.tensor.matmul(
                                        ps2[0:64, :], W[cb][:, 128:192], xnT[cb][:, sl],
                                        start=(cb == 0), stop=(cb == NC - 1),
                                    )
                                nc.vector.tensor_scalar_add(
                                    dst_c[:, sl], ps2[0:64, :], bias[0:64, 1:2])

                        for tb in range(NT):
                            nc.vector.memset(Vbf[tb][:, :, DH:DH + 1], 1.0)
                            psv = psA.tile([128, 512], f32, tag="psA", name="psv")
                            for cb in range(NC):
                                nc.tensor.matmul(
                                    psv[:, 0:HL * DH],
                                    xnT[cb][:, tb * 128:(tb + 1) * 128],
                                    wv[cb][:],
                                    start=(cb == 0), stop=False,
                                )
                            nc.tensor.matmul(
                                psv[:, 0:HL * DH], ones_col[:], bvs[:],
                                start=False, stop=True,
                            )
                            nc.scalar.copy(
                                Vbf[tb][:, :, 0:DH],
                                psv[:, 0:HL * DH].rearrange(
                                    "p (h d) -> p h d", d=DH),
                            )

                # ===== Phase C: attention (3 heads, full causal) =====
                qslices = (
                    (QT_ab, 0), (QT_ab, 64), (QT_c, 0),
                )
                kslices = (
                    (KT_ab, 0), (KT_ab, 64), (KT_c, 0),
                )

                with (
                    tc.tile_pool(name="exps", bufs=1) as expp,
                    tc.tile_pool(name="ysb", bufs=8) as ysbp,
                    tc.tile_pool(name="pad", bufs=4) as padp,
                    tc.tile_pool(name="yT", bufs=3) as yTp,
                ):
                    # DRAM bounce buffers for the two ReduceScatters
                    ccA_i = dramp.tile([1536, 4 * HL * (DH + 1)], bf16, name="ccA_i")
                    ccA_o = dramp.tile([384, 4 * HL * (DH + 1)], bf16, name="ccA_o")
                    ccB_i = dramp.tile([512, 4 * HL * (DH + 1)], bf16, name="ccB_i")
                    ccB_o = dramp.tile([128, 4 * HL * (DH + 1)], bf16, name="ccB_o")

                    # expST tiles allocated per (ci, h) below
                    for ci in range(4):
                        i0 = ci * 512
                        nj = 4 * (ci + 1)
                        exps = {}
                        for h in range(HL):
                            qt, qoff = qslices[h]
                            kt, koff = kslices[h]
                            for jb in range(nj):
                                et = expp.tile([128, 512], bf16, tag=f"e{h}_{jb}",
                                               name=f"e{h}_{jb}")
                                exps[(h, jb)] = et
                                r = jb - 4 * ci
                                off = 128 * r if r >= 0 else 0
                                w = 512 - off
                                pss = psA.tile([128, 512], f32, tag="psA",
                                               name="pss")
                                nc.tensor.matmul(
                                    pss[:, 0:w],
                                    qt[qoff:qoff + 64, jb * 128:(jb + 1) * 128],
                                    kt[koff:koff + 64, i0 + off:i0 + 512],
                                    start=True, stop=True,
                                )
                                if r >= 0:
                                    nc.vector.tensor_add(
                                        pss[:, 0:128], pss[:, 0:128], negtri[:])
                                    if off:
                                        nc.vector.memset(et[:, 0:off], 0.0)
                                nc.scalar.activation(
                                    et[:, off:512], pss[:, 0:w],
                                    AF.Exp, scale=0.125,
                                )
                        # P@V per head (bf16, accumulate over j-blocks)
                        psys = []
                        for h in range(HL):
                            psy = psPV.tile([128, 512], f32, tag="psPV", name="psy")
                            psys.append(psy)
                            for jb in range(nj):
                                nc.tensor.matmul(
                                    psy[0:DH + 1, :],
                                    Vbf[jb][:, h, :],
                                    exps[(h, jb)][:],
                                    start=(jb == 0), stop=(jb == nj - 1),
                                )
                        # transpose yT -> token-major y_sb tiles
                        y_sb = [ysbp.tile([128, HL * (DH + 1)], bf16, tag="ysb",
                                          name=f"ysb{ib}") for ib in range(4)]
                        for h in range(HL):
                            yT_bf = yTp.tile([128, 512], bf16, tag="yT", name="yT")
                            nc.vector.tensor_copy(yT_bf[0:DH + 1, :],
                                                  psys[h][0:DH + 1, :])
                            for ib in range(4):
                                tp = psTr.tile([128, 128], bf16, tag="psTr",
                                               name="tpy")
                                nc.tensor.matmul(
                                    tp[:], yT_bf[:, ib * 128:(ib + 1) * 128],
                                    ident[:], is_transpose=True,
                                    start=True, stop=True,
                                )
                                nc.vector.tensor_copy(
                                    y_sb[ib][:, h * (DH + 1):(h + 1) * (DH + 1)],
                                    tp[:, 0:DH + 1])
                        # pad into 4 receiver slots and ship to the RS buffer
                        W780 = 4 * HL * (DH + 1)
                        for ib in range(4):
                            pad = padp.tile([128, 4, HL * (DH + 1)], bf16, tag="pad",
                                            name="pad")
                            for t in range(4):
                                nc.gpsimd.tensor_scalar_mul(
                                    pad[:, t, :], y_sb[ib][:], msel[:, t:t + 1])
                            if ci < 3:
                                # RS-A block ib covers rows {512*cj + 128*ib}
                                nc.gpsimd.dma_start(
                                    ccA_i[ib * 384 + ci * 128:
                                          ib * 384 + ci * 128 + 128, :],
                                    pad[:].rearrange("p t d -> p (t d)"),
                                )
                            else:
                                nc.gpsimd.dma_start(
                                    ccB_i[ib * 128:(ib + 1) * 128, :],
                                    pad[:].rearrange("p t d -> p (t d)"),
                                )
                        if ci == 2:
                            nc.gpsimd.collective_compute(
                                "ReduceScatter",
                                mybir.AluOpType.add,
                                replica_groups=[[0, 1, 2, 3], [4, 5, 6, 7]],
                                ins=[ccA_i[:]],
                                outs=[ccA_o[:]],
                            )
                        if ci == 3:
                            nc.gpsimd.collective_compute(
                                "ReduceScatter",
                                mybir.AluOpType.add,
                                replica_groups=[[0, 1, 2, 3], [4, 5, 6, 7]],
                                ins=[ccB_i[:]],
                                outs=[ccB_o[:]],
                            )

            # ===== Phase D/E: readback, residual, LN2, MLP =====
            W65 = DH + 1
            W195 = HL * W65
            with (
                tc.tile_pool(name="yin", bufs=2) as yinp,
                tc.tile_pool(name="x1p", bufs=1) as x1p,
                tc.tile_pool(name="x1nT", bufs=1) as x1nTp,
                tc.tile_pool(name="w1p", bufs=1) as w1p,
                tc.tile_pool(name="w2p", bufs=1) as w2p,
                tc.tile_pool(name="h1T", bufs=1) as h1Tp,
                tc.tile_pool(name="stat2", bufs=4) as stat2p,
                tc.tile_pool(name="dtmp", bufs=4) as dtmpp,
                tc.tile_pool(name="outp", bufs=2) as outp,
            ):
                w1 = [w1p.tile([128, F], bf16, name=f"w1_{cb}") for cb in range(NC)]
                for cb in range(NC):
                    nc.sync.dma_start(w1[cb][:], w1_d[cb * 128:(cb + 1) * 128, :])
                w2 = [w2p.tile([128, C], bf16, name=f"w2_{nb}") for nb in range(NF)]
                for nb in range(NF):
                    nc.sync.dma_start(w2[nb][:], w2_d[nb * 128:(nb + 1) * 128, :])
                b2s = w2p.tile([1, C], bf16, name="b2s")
                nc.sync.dma_start(b2s[:], b2_d[:])

                x1 = [x1p.tile([128, C], f32, name=f"x1_{k}") for k in range(4)]
                x1nT = [x1nTp.tile([128, 512], bf16, name=f"x1nT{cb}")
                        for cb in range(NC)]
                h1T = [h1Tp.tile([128, 512], bf16, name=f"h1T{nb}")
                       for nb in range(NF)]

                def strip(k, cc_o, row0):
                    """x1[k] = sel(xn)*g1 + b1r + y/denom; LN2 -> x1nT cols."""
                    yall = yinp.tile([128, 4 * W195], bf16, tag="yin", name="yin")
                    nc.sync.dma_start(yall[:], cc_o[row0:row0 + 128, :])
                    yv = yall[:].rearrange("p (q d) -> p q d", d=W65)
                    rec = dtmpp.tile([128, H], f32, tag="rec", name="rec")
                    nc.vector.tensor_copy(rec[:], yv[:, :, DH])
                    nc.vector.reciprocal(rec[:], rec[:])
                    yf = dtmpp.tile([128, C], f32, tag="yf", name="yf")
                    for hh in range(H):
                        nc.gpsimd.tensor_scalar_mul(
                            yf[:, hh * DH:(hh + 1) * DH],
                            yv[:, hh, 0:DH],
                            rec[:, hh:hh + 1],
                        )
                    # select own xn rows via one-hot identity matmul
                    for gg in range(2):
                        psx = psA.tile([128, 512], f32, tag="psA", name="psx")
                        for t in range(4):
                            nc.tensor.matmul(
                                psx[:, 0:384],
                                selid[:, t * 128:(t + 1) * 128],
                                xn_bf[4 * k + t][:, gg * 384:(gg + 1) * 384],
                                start=(t == 0), stop=(t == 3),
                            )
                        gs = slice(gg * 384, (gg + 1) * 384)
                        nc.vector.tensor_tensor(
                            x1[k][:, gs], psx[:, 0:384], g1b[:, gs], ALU.mult)
                        nc.vector.tensor_add(x1[k][:, gs], x1[k][:, gs], b1rb[:, gs])
                        nc.vector.tensor_add(x1[k][:, gs], x1[k][:, gs], yf[:, gs])
                    # LN2
                    st6 = stat2p.tile([128, 2, 6], f32, tag="st6", name="st6b")
                    for gg in range(2):
                        nc.vector.bn_stats(
                            st6[:, gg, :], x1[k][:, gg * 384:(gg + 1) * 384])
                    st2 = stat2p.tile([128, 2], f32, tag="st2", name="st2b")
                    nc.vector.bn_aggr(st2[:], st6[:])
                    std = stat2p.tile([128, 1], f32, tag="std", name="stdb")
                    nc.scalar.activation(std[:], st2[:, 1:2], AF.Sqrt, bias=eps_t[:])
                    rstd = stat2p.tile([128, 1], f32, tag="rstd", name="rstdb")
                    nc.vector.reciprocal(rstd[:], std[:])
                    nmb = stat2p.tile([128, 1], f32, tag="nmb", name="nmbb")
                    nc.vector.tensor_scalar(
                        nmb[:], st2[:, 0:1], rstd[:], -1.0,
                        op0=ALU.mult, op1=ALU.mult,
                    )
                    x1n = dtmpp.tile([128, C], bf16, tag="x1n", name="x1n")
                    nc.scalar.activation(
                        x1n[:], x1[k][:], AF.Identity, bias=nmb[:], scale=rstd[:])
                    for cb in range(NC):
                        tp = psTr.tile([128, 128], bf16, tag="psTr", name="tpb")
                        nc.tensor.matmul(
                            tp[:], x1n[:, cb * 128:(cb + 1) * 128],
                            ident[:], is_transpose=True, start=True, stop=True,
                        )
                        if cb % 2 == 0:
                            nc.scalar.copy(x1nT[cb][:, k * 128:(k + 1) * 128], tp[:])
                        else:
                            nc.vector.tensor_copy(
                                x1nT[cb][:, k * 128:(k + 1) * 128], tp[:])

                def mlp(col0, w):
                    """h1T[:, col0:col0+w] = gelu(W1^T x1nT + b1)."""
                    for nb in range(NF):
                        ps = psA.tile([128, 512], f32, tag="psA", name="psh")
                        for cb in range(NC):
                            nc.tensor.matmul(
                                ps[:, 0:w],
                                w1[cb][:, nb * 128:(nb + 1) * 128],
                                x1nT[cb][:, col0:col0 + w],
                                start=(cb == 0), stop=(cb == NC - 1),
                            )
                        nc.scalar.activation(
                            h1T[nb][:, col0:col0 + w], ps[:, 0:w],
                            AF.Gelu, bias=b1s[:, nb:nb + 1],
                        )

                def outproj(k):
                    o_sb = outp.tile([128, C], f32, tag="o", name="o_sb")
                    for gg in range(2):
                        ps = psA.tile([128, 512], f32, tag="psA", name="pso")
                        for nb in range(NF):
                            nc.tensor.matmul(
                                ps[:, 0:384],
                                h1T[nb][:, k * 128:(k + 1) * 128],
                                w2[nb][:, gg * 384:(gg + 1) * 384],
                                start=(nb == 0), stop=False,
                            )
                        nc.tensor.matmul(
                            ps[:, 0:384], ones_col[:],
                            b2s[:, gg * 384:(gg + 1) * 384],
                            start=False, stop=True,
                        )
                        nc.vector.tensor_add(
                            o_sb[:, gg * 384:(gg + 1) * 384], ps[:, 0:384],
                            x1[k][:, gg * 384:(gg + 1) * 384],
                        )
                    nc.sync.dma_start(out_d[k * 128:(k + 1) * 128, :], o_sb[:])

                for k in range(3):
                    strip(k, ccA_o, k * 128)
                mlp(0, 384)
                for k in range(3):
                    outproj(k)
                strip(3, ccB_o, 0)
                mlp(384, 128)
                outproj(3)

    nc.compile()
    return nc


def _prep_inputs(inputs):
    import ml_dtypes

    f = np.float32
    bf = ml_dtypes.bfloat16
    g1 = np.asarray(inputs["ln1_g"], f)
    b1r = np.asarray(inputs["ln1_b"], f)
    g2 = np.asarray(inputs["ln2_g"], f)
    b2r = np.asarray(inputs["ln2_b"], f)
    Wq, Wk, Wv = (np.asarray(inputs[k], f) for k in ("Wq", "Wk", "Wv"))
    W1, W2 = np.asarray(inputs["W1"], f), np.asarray(inputs["W2"], f)
    x = np.asarray(inputs["x"], f)

    def c(a, dtype=bf):
        return np.ascontiguousarray(a.astype(dtype))

    wq_f = g1[:, None] * Wq
    wk_f = g1[:, None] * Wk
    wv_f = g1[:, None] * Wv
    bq_f = b1r @ Wq + np.asarray(inputs["bq"], f)
    bk_f = b1r @ Wk + np.asarray(inputs["bk"], f)
    bv_f = b1r @ Wv + np.asarray(inputs["bv"], f)
    b1_f = b2r @ W1 + np.asarray(inputs["b1"], f)

    shared = {
        "w1": c(g2[:, None] * W1),
        "b1": np.ascontiguousarray(b1_f.reshape(NF, 128).T).astype(f),
        "w2": c(W2),
        "b2": c(np.asarray(inputs["b2"], f)[None, :]),
        "g1": c(g1[None, :]),
        "b1r": c(b1r[None, :]),
        "negtri": np.ascontiguousarray(
            NEG * np.tril(np.ones((128, 128), f), -1)).astype(f),
        "ident": c(np.eye(128, dtype=f)),
    }

    def bias2(b):
        out = np.zeros((128, 2), f)
        out[:, 0] = b[0:128]
        out[0:64, 1] = b[128:192]
        return out

    in_maps = []
    for core in range(8):
        b, g = core // 4, core % 4
        cols = slice(192 * g, 192 * (g + 1))
        msel = np.zeros((128, 4), f)
        msel[:, g] = 1.0
        selid = np.zeros((128, 512), f)
        selid[:, 128 * g:128 * (g + 1)] = np.eye(128, dtype=f)
        m = dict(shared)
        m["x"] = c(x[b])
        m["wq"] = c(wq_f[:, cols])
        m["wk"] = c(wk_f[:, cols])
        m["wv"] = c(wv_f[:, cols])
        m["bq"] = bias2(bq_f[cols])
        m["bk"] = bias2(bk_f[cols])
        m["bv"] = c(bv_f[cols][None, :])
        m["msel"] = msel
        m["selid"] = c(selid)
        in_maps.append(m)
    return in_maps


def kernel(**inputs):
    from concourse.bass_utils import run_bass_kernel_spmd

    if "nc" not in _CACHE:
        _CACHE["nc"] = _build_program()
    nc = _CACHE["nc"]

    in_maps = _prep_inputs(inputs)

    trace = bool(int(os.environ.get("KERNEL_TRACE", "0")))
    try:
        res = run_bass_kernel_spmd(nc, in_maps, core_ids=list(range(8)), trace=trace)
    except ModuleNotFoundError:
        res = run_bass_kernel_spmd(nc, in_maps, core_ids=list(range(8)), trace=False)
    _CACHE["last_result"] = res

    out = np.empty((B, T, C), np.float32)
    for core in range(8):
        b, g = core // 4, core % 4
        r = res.results[core]["out"]
        for k in range(4):
            out[b, 512 * k + 128 * g:512 * k + 128 * g + 128] = \
                r[128 * k:128 * (k + 1)]
    return out


# revision 11
# speedup vs baseline: 1.2339x; 1.2339x over previous
"""Trainium2 Bass kernel for a GPT-style decoder block (B=2, T=2048, C=768, H=12).

Sharding v2: 8 cores = 2 batches x 4 head-groups. Core (b, g) runs attention
for heads {3g, 3g+1, 3g+2} over ALL T rows (full causal triangle), then the
cores of a batch exchange attention outputs with two bf16 ReduceScatters so
each core runs LN2+MLP on a disjoint block-cyclic quarter of the rows
(strips rows [512*ci + 128*g, +128) for ci in 0..3).

The SPMD program is identical on every core; all core-dependence lives in the
inputs: sliced QKV weights, plus one-hot select tensors (msel / selident) that
route data into the right ReduceScatter slot and select the core's own rows
out of replicated LN1 activations.

Attention layout: scoresT[j, i] = k_i . q_j (reference computes K @ Q^T), so
the moving dim of the score matmuls is i (own-output tokens) and softmax sums
over partitions j via a ones-column that rides in V. P@V runs in fp8e4 with
MatmulPerfMode.DoubleRow (2x PE throughput); everything else is bf16 with
fp32 accumulation. Causality: score matmuls only cover i >= j-block start;
the diagonal 128-block gets -1e30 added pre-exp; sub-diagonal exp slots are
memset to 0.

LN gamma/beta are folded into the adjacent matmul weights/biases host-side.
"""

import os

import numpy as np

B, T, C = 2, 2048, 768
H, DH = 12, 64
HL = 3               # heads per core
F = 4 * C
NT = T // 128        # 16 token tiles
NC = C // 128        # 6 channel chunks
NF = F // 128        # 24 hidden chunks
EPS = 1e-3
NEG = -1e30

_CACHE = {}


def _build_program():
    import concourse.bass as bass  # noqa: F401
    import concourse.mybir as mybir
    import concourse.tile as tile
    from concourse import bacc

    dt = mybir.dt
    f32 = dt.float32
    bf16 = dt.bfloat16
    f8 = dt.float8e4
    AF = mybir.ActivationFunctionType
    ALU = mybir.AluOpType
    DR = mybir.MatmulPerfMode.DoubleRow

    nc = bacc.Bacc("TRN2", target_bir_lowering=False, debug=False, num_devices=8)

    # ---- DRAM I/O ----
    x_d = nc.dram_tensor("x", [T, C], bf16, kind="ExternalInput")
    wq_d = nc.dram_tensor("wq", [C, HL * DH], bf16, kind="ExternalInput")
    wk_d = nc.dram_tensor("wk", [C, HL * DH], bf16, kind="ExternalInput")
    wv_d = nc.dram_tensor("wv", [C, HL * DH], bf16, kind="ExternalInput")
    bq_d = nc.dram_tensor("bq", [128, 2], f32, kind="ExternalInput")
    bk_d = nc.dram_tensor("bk", [128, 2], f32, kind="ExternalInput")
    bv_d = nc.dram_tensor("bv", [1, HL * DH], bf16, kind="ExternalInput")
    w1_d = nc.dram_tensor("w1", [C, F], bf16, kind="ExternalInput")
    b1_d = nc.dram_tensor("b1", [128, NF], f32, kind="ExternalInput")
    w2_d = nc.dram_tensor("w2", [F, C], bf16, kind="ExternalInput")
    b2_d = nc.dram_tensor("b2", [1, C], bf16, kind="ExternalInput")
    g1_d = nc.dram_tensor("g1", [1, C], bf16, kind="ExternalInput")
    b1r_d = nc.dram_tensor("b1r", [1, C], bf16, kind="ExternalInput")
    negtri_d = nc.dram_tensor("negtri", [128, 128], f32, kind="ExternalInput")
    ident_d = nc.dram_tensor("ident", [128, 128], bf16, kind="ExternalInput")
    msel_d = nc.dram_tensor("msel", [128, 4], f32, kind="ExternalInput")
    selid_d = nc.dram_tensor("selid", [128, 512], bf16, kind="ExternalInput")
    out_d = nc.dram_tensor("out", [512, C], f32, kind="ExternalOutput")

    with tile.TileContext(nc) as tc:
        with (
            tc.tile_pool(name="const", bufs=1) as constp,
            tc.tile_pool(name="dram", bufs=1, space="DRAM") as dramp,
            tc.tile_pool(name="psA", bufs=2, space="PSUM") as psA,
            tc.tile_pool(name="psPV", bufs=3, space="PSUM") as psPV,
            tc.tile_pool(name="psTr", bufs=2, space="PSUM") as psTr,
        ):
            # ---- constants ----
            negtri = constp.tile([128, 128], f32)
            nc.sync.dma_start(negtri[:], negtri_d[:])
            ident = constp.tile([128, 128], bf16)
            nc.sync.dma_start(ident[:], ident_d[:])
            msel = constp.tile([128, 4], f32)
            nc.sync.dma_start(msel[:], msel_d[:])
            selid = constp.tile([128, 512], bf16)
            nc.sync.dma_start(selid[:], selid_d[:])
            bqs = constp.tile([128, 2], f32)
            nc.sync.dma_start(bqs[:], bq_d[:])
            bks = constp.tile([128, 2], f32)
            nc.sync.dma_start(bks[:], bk_d[:])
            b1s = constp.tile([128, NF], f32)
            nc.sync.dma_start(b1s[:], b1_d[:])
            ones_col = constp.tile([1, 128], bf16)
            nc.vector.memset(ones_col[:], 1.0)
            eps_t = constp.tile([128, 1], f32)
            nc.vector.memset(eps_t[:], EPS)

            # broadcast ln1 gamma/beta rows to [128, C] via rank-1 matmuls
            g1s = constp.tile([1, C], bf16)
            nc.sync.dma_start(g1s[:], g1_d[:])
            b1rs = constp.tile([1, C], bf16)
            nc.sync.dma_start(b1rs[:], b1r_d[:])
            g1b = constp.tile([128, C], f32)
            b1rb = constp.tile([128, C], f32)
            for dst, src in ((g1b, g1s), (b1rb, b1rs)):
                for gg in range(2):
                    ps = psA.tile([128, 512], f32, tag="psA", name="psbc")
                    nc.tensor.matmul(
                        ps[:, 0:384], ones_col[:], src[:, gg * 384:(gg + 1) * 384],
                        start=True, stop=True,
                    )
                    nc.vector.tensor_copy(dst[:, gg * 384:(gg + 1) * 384], ps[:, 0:384])

            # persistent activations
            xn_bf = [constp.tile([128, C], bf16, name=f"xnbf{tb}") for tb in range(NT)]

            with (
                tc.tile_pool(name="QK", bufs=1) as qkp,
                tc.tile_pool(name="V8", bufs=1) as v8p,
            ):
                QT_ab = qkp.tile([128, T], bf16, name="QT_ab")
                QT_c = qkp.tile([64, T], bf16, name="QT_c")
                KT_ab = qkp.tile([128, T], bf16, name="KT_ab")
                KT_c = qkp.tile([64, T], bf16, name="KT_c")
                Vbf = [v8p.tile([128, HL, DH + 1], bf16, name=f"V_{tb}")
                       for tb in range(NT)]

                with tc.tile_pool(name="xnT", bufs=1) as xnTp:
                    xnT = [xnTp.tile([128, T], bf16, name=f"xnT{cb}") for cb in range(NC)]

                    # ===== Phase A: LN1 over all T + transpose to xnT =====
                    with (
                        tc.tile_pool(name="xin", bufs=3) as xinp,
                        tc.tile_pool(name="stat", bufs=4) as statp,
                    ):
                        for tb in range(NT):
                            xt = xinp.tile([128, C], bf16, tag="xt", name="xt")
                            nc.sync.dma_start(xt[:], x_d[tb * 128:(tb + 1) * 128, :])
                            st6 = statp.tile([128, 2, 6], f32, tag="st6", name="st6")
                            for gg in range(2):
                                nc.vector.bn_stats(
                                    st6[:, gg, :], xt[:, gg * 384:(gg + 1) * 384]
                                )
                            st2 = statp.tile([128, 2], f32, tag="st2", name="st2")
                            nc.vector.bn_aggr(st2[:], st6[:])
                            std = statp.tile([128, 1], f32, tag="std", name="std")
                            nc.scalar.activation(std[:], st2[:, 1:2], AF.Sqrt, bias=eps_t[:])
                            rstd = statp.tile([128, 1], f32, tag="rstd", name="rstd")
                            nc.vector.reciprocal(rstd[:], std[:])
                            nmb = statp.tile([128, 1], f32, tag="nmb", name="nmb")
                            nc.vector.tensor_scalar(
                                nmb[:], st2[:, 0:1], rstd[:], -1.0,
                                op0=ALU.mult, op1=ALU.mult,
                            )
                            nc.scalar.activation(
                                xn_bf[tb][:], xt[:], AF.Identity,
                                bias=nmb[:], scale=rstd[:],
                            )
                            for cb in range(NC):
                                tp = psTr.tile([128, 128], bf16, tag="psTr", name="tp")
                                nc.tensor.matmul(
                                    tp[:], xn_bf[tb][:, cb * 128:(cb + 1) * 128],
                                    ident[:], is_transpose=True, start=True, stop=True,
                                )
                                if cb % 2 == 0:
                                    nc.scalar.copy(
                                        xnT[cb][:, tb * 128:(tb + 1) * 128], tp[:])
                                else:
                                    nc.vector.tensor_copy(
                                        xnT[cb][:, tb * 128:(tb + 1) * 128], tp[:])

                    # ===== Phase B: QKV projections (3 heads) =====
                    with tc.tile_pool(name="wqkv", bufs=1) as wp:
                        wq = [wp.tile([128, HL * DH], bf16, name=f"wq{cb}") for cb in range(NC)]
                        wk = [wp.tile([128, HL * DH], bf16, name=f"wk{cb}") for cb in range(NC)]
                        wv = [wp.tile([128, HL * DH], bf16, name=f"wv{cb}") for cb in range(NC)]
                        for cb in range(NC):
                            nc.sync.dma_start(wq[cb][:], wq_d[cb * 128:(cb + 1) * 128, :])
                            nc.sync.dma_start(wk[cb][:], wk_d[cb * 128:(cb + 1) * 128, :])
                            nc.sync.dma_start(wv[cb][:], wv_d[cb * 128:(cb + 1) * 128, :])
                        bvs = wp.tile([1, HL * DH], bf16, name="bvs")
                        nc.sync.dma_start(bvs[:], bv_d[:])

                        for jb in range(T // 512):
                            sl = slice(jb * 512, (jb + 1) * 512)
                            for (W, dst_ab, dst_c, bias) in (
                                (wq, QT_ab, QT_c, bqs),
                                (wk, KT_ab, KT_c, bks),
                            ):
                                ps = psA.tile([128, 512], f32, tag="psA", name="psq")
                                for cb in range(NC):
                                    nc.tensor.matmul(
                                        ps[:], W[cb][:, 0:128], xnT[cb][:, sl],
                                        start=(cb == 0), stop=(cb == NC - 1),
                                    )
                                nc.vector.tensor_scalar_add(
                                    dst_ab[:, sl], ps[:], bias[:, 0:1])
                                ps2 = psA.tile([128, 512], f32, tag="psA", name="psq2")
                                for cb in range(NC):
                                    nc.tensor.matmul(
                                        ps2[0:64, :], W[cb][:, 128:192], xnT[cb][:, sl],
                                        start=(cb == 0), stop=(cb == NC - 1),
                                    )
                                nc.vector.tensor_scalar_add(
                                    dst_c[:, sl], ps2[0:64, :], bias[0:64, 1:2])

                        for tb in range(NT):
                            nc.vector.memset(Vbf[tb][:, :, DH:DH + 1], 1.0)
                            psv = psA.tile([128, 512], f32, tag="psA", name="psv")
                            for cb in range(NC):
                                nc.tensor.matmul(
                                    psv[:, 0:HL * DH],
                                    xnT[cb][:, tb * 128:(tb + 1) * 128],
                                    wv[cb][:],
                                    start=(cb == 0), stop=False,
                                )
                            nc.tensor.matmul(
                                psv[:, 0:HL * DH], ones_col[:], bvs[:],
                                start=False, stop=True,
                            )
                            nc.scalar.copy(
                                Vbf[tb][:, :, 0:DH],
                                psv[:, 0:HL * DH].rearrange(
                                    "p (h d) -> p h d", d=DH),
                            )

                # ===== Phase C: attention (3 heads, full causal) =====
                qslices = (
                    (QT_ab, 0), (QT_ab, 64), (QT_c, 0),
                )
                kslices = (
                    (KT_ab, 0), (KT_ab, 64), (KT_c, 0),
                )

                with (
                    tc.tile_pool(name="exps", bufs=1) as expp,
                    tc.tile_pool(name="ysb", bufs=8) as ysbp,
                    tc.tile_pool(name="pad", bufs=4) as padp,
                    tc.tile_pool(name="yT", bufs=3) as yTp,
                ):
                    # DRAM bounce buffers for the two ReduceScatters
                    ccA_i = dramp.tile([1536, 4 * HL * (DH + 1)], bf16, name="ccA_i")
                    ccA_o = dramp.tile([384, 4 * HL * (DH + 1)], bf16, name="ccA_o")
                    ccB_i = dramp.tile([512, 4 * HL * (DH + 1)], bf16, name="ccB_i")
                    ccB_o = dramp.tile([128, 4 * HL * (DH + 1)], bf16, name="ccB_o")

                    # expST tiles allocated per (ci, h) below
                    for ci in range(4):
                        i0 = ci * 512
                        nj = 4 * (ci + 1)
                        exps = {}
                        for h in range(HL):
                            qt, qoff = qslices[h]
                            kt, koff = kslices[h]
                            for jb in range(nj):
                                et = expp.tile([128, 512], bf16, tag=f"e{h}_{jb}",
                                               name=f"e{h}_{jb}")
                                exps[(h, jb)] = et
                                r = jb - 4 * ci
                                off = 128 * r if r >= 0 else 0
                                w = 512 - off
                                pss = psA.tile([128, 512], f32, tag="psA",
                                               name="pss")
                                nc.tensor.matmul(
                                    pss[:, 0:w],
                                    qt[qoff:qoff + 64, jb * 128:(jb + 1) * 128],
                                    kt[koff:koff + 64, i0 + off:i0 + 512],
                                    start=True, stop=True,
                                )
                                if r >= 0:
                                    nc.vector.tensor_add(
                                        pss[:, 0:128], pss[:, 0:128], negtri[:])
                                    if off:
                                        nc.vector.memset(et[:, 0:off], 0.0)
                                nc.scalar.activation(
                                    et[:, off:512], pss[:, 0:w],
                                    AF.Exp, scale=0.125,
                                )
                        # P@V per head (bf16, accumulate over j-blocks)
                        psys = []
                        for h in range(HL):
                            psy = psPV.tile([128, 512], f32, tag="psPV", name="psy")
                            psys.append(psy)
                            for jb in range(nj):
                                nc.tensor.matmul(
                                    psy[0:DH + 1, :],
                                    Vbf[jb][:, h, :],
                                    exps[(h, jb)][:],
                                    start=(jb == 0), stop=(jb == nj - 1),
                                )
                        # transpose yT -> token-major y_sb tiles
                        y_sb = [ysbp.tile([128, HL * (DH + 1)], bf16, tag="ysb",
                                          name=f"ysb{ib}") for ib in range(4)]
                        for h in range(HL):
                            yT_bf = yTp.tile([128, 512], bf16, tag="yT", name="yT")
                            nc.vector.tensor_copy(yT_bf[0:DH + 1, :],
                                                  psys[h][0:DH + 1, :])
                            for ib in range(4):
                                tp = psTr.tile([128, 128], bf16, tag="psTr",
                                               name="tpy")
                                nc.tensor.matmul(
                                    tp[:], yT_bf[:, ib * 128:(ib + 1) * 128],
                                    ident[:], is_transpose=True,
                                    start=True, stop=True,
                                )
                                nc.vector.tensor_copy(
                                    y_sb[ib][:, h * (DH + 1):(h + 1) * (DH + 1)],
                                    tp[:, 0:DH + 1])
                        # pad into 4 receiver slots and ship to the RS buffer
                        W780 = 4 * HL * (DH + 1)
                        for ib in range(4):
                            pad = padp.tile([128, 4, HL * (DH + 1)], bf16, tag="pad",
                                            name="pad")
                            for t in range(4):
                                nc.vector.tensor_scalar_mul(
                                    pad[:, t, :], y_sb[ib][:], msel[:, t:t + 1])
                            if ci < 3:
                                # RS-A block ib covers rows {512*cj + 128*ib}
                                nc.gpsimd.dma_start(
                                    ccA_i[ib * 384 + ci * 128:
                                          ib * 384 + ci * 128 + 128, :],
                                    pad[:].rearrange("p t d -> p (t d)"),
                                )
                            else:
                                nc.gpsimd.dma_start(
                                    ccB_i[ib * 128:(ib + 1) * 128, :],
                                    pad[:].rearrange("p t d -> p (t d)"),
                                )
                        if ci == 2:
                            nc.gpsimd.collective_compute(
                                "ReduceScatter",
                                mybir.AluOpType.add,
                                replica_groups=[[0, 1, 2, 3], [4, 5, 6, 7]],
                                ins=[ccA_i[:]],
                                outs=[ccA_o[:]],
                            )
                        if ci == 3:
                            nc.gpsimd.collective_compute(
                                "ReduceScatter",
                                mybir.AluOpType.add,
                                replica_groups=[[0, 1, 2, 3], [4, 5, 6, 7]],
                                ins=[ccB_i[:]],
                                outs=[ccB_o[:]],
                            )

            # ===== Phase D/E: readback, residual, LN2, MLP =====
            W65 = DH + 1
            W195 = HL * W65
            with (
                tc.tile_pool(name="yin", bufs=2) as yinp,
                tc.tile_pool(name="x1p", bufs=1) as x1p,
                tc.tile_pool(name="x1nT", bufs=1) as x1nTp,
                tc.tile_pool(name="w1p", bufs=1) as w1p,
                tc.tile_pool(name="w2p", bufs=1) as w2p,
                tc.tile_pool(name="h1T", bufs=1) as h1Tp,
                tc.tile_pool(name="stat2", bufs=4) as stat2p,
                tc.tile_pool(name="dtmp", bufs=4) as dtmpp,
                tc.tile_pool(name="outp", bufs=2) as outp,
            ):
                w1 = [w1p.tile([128, F], bf16, name=f"w1_{cb}") for cb in range(NC)]
                for cb in range(NC):
                    nc.sync.dma_start(w1[cb][:], w1_d[cb * 128:(cb + 1) * 128, :])
                w2 = [w2p.tile([128, C], bf16, name=f"w2_{nb}") for nb in range(NF)]
                for nb in range(NF):
                    nc.sync.dma_start(w2[nb][:], w2_d[nb * 128:(nb + 1) * 128, :])
                b2s = w2p.tile([1, C], bf16, name="b2s")
                nc.sync.dma_start(b2s[:], b2_d[:])

                x1 = [x1p.tile([128, C], f32, name=f"x1_{k}") for k in range(4)]
                x1nT = [x1nTp.tile([128, 512], bf16, name=f"x1nT{cb}")
                        for cb in range(NC)]
                h1T = [h1Tp.tile([128, 512], bf16, name=f"h1T{nb}")
                       for nb in range(NF)]

                def strip(k, cc_o, row0):
                    """x1[k] = sel(xn)*g1 + b1r + y/denom; LN2 -> x1nT cols."""
                    yall = yinp.tile([128, 4 * W195], bf16, tag="yin", name="yin")
                    nc.sync.dma_start(yall[:], cc_o[row0:row0 + 128, :])
                    yv = yall[:].rearrange("p (q d) -> p q d", d=W65)
                    rec = dtmpp.tile([128, H], f32, tag="rec", name="rec")
                    nc.vector.tensor_copy(rec[:], yv[:, :, DH])
                    nc.vector.reciprocal(rec[:], rec[:])
                    yf = dtmpp.tile([128, C], f32, tag="yf", name="yf")
                    for hh in range(H):
                        nc.vector.tensor_scalar_mul(
                            yf[:, hh * DH:(hh + 1) * DH],
                            yv[:, hh, 0:DH],
                            rec[:, hh:hh + 1],
                        )
                    # select own xn rows via one-hot identity matmul
                    for gg in range(2):
                        psx = psA.tile([128, 512], f32, tag="psA", name="psx")
                        for t in range(4):
                            nc.tensor.matmul(
                                psx[:, 0:384],
                                selid[:, t * 128:(t + 1) * 128],
                                xn_bf[4 * k + t][:, gg * 384:(gg + 1) * 384],
                                start=(t == 0), stop=(t == 3),
                            )
                        gs = slice(gg * 384, (gg + 1) * 384)
                        nc.vector.tensor_tensor(
                            x1[k][:, gs], psx[:, 0:384], g1b[:, gs], ALU.mult)
                        nc.vector.tensor_add(x1[k][:, gs], x1[k][:, gs], b1rb[:, gs])
                        nc.vector.tensor_add(x1[k][:, gs], x1[k][:, gs], yf[:, gs])
                    # LN2
                    st6 = stat2p.tile([128, 2, 6], f32, tag="st6", name="st6b")
                    for gg in range(2):
                        nc.vector.bn_stats(
                            st6[:, gg, :], x1[k][:, gg * 384:(gg + 1) * 384])
                    st2 = stat2p.tile([128, 2], f32, tag="st2", name="st2b")
                    nc.vector.bn_aggr(st2[:], st6[:])
                    std = stat2p.tile([128, 1], f32, tag="std", name="stdb")
                    nc.scalar.activation(std[:], st2[:, 1:2], AF.Sqrt, bias=eps_t[:])
                    rstd = stat2p.tile([128, 1], f32, tag="rstd", name="rstdb")
                    nc.vector.reciprocal(rstd[:], std[:])
                    nmb = stat2p.tile([128, 1], f32, tag="nmb", name="nmbb")
                    nc.vector.tensor_scalar(
                        nmb[:], st2[:, 0:1], rstd[:], -1.0,
                        op0=ALU.mult, op1=ALU.mult,
                    )
                    x1n = dtmpp.tile([128, C], bf16, tag="x1n", name="x1n")
                    nc.scalar.activation(
                        x1n[:], x1[k][:], AF.Identity, bias=nmb[:], scale=rstd[:])
                    for cb in range(NC):
                        tp = psTr.tile([128, 128], bf16, tag="psTr", name="tpb")
                        nc.tensor.matmul(
                            tp[:], x1n[:, cb * 128:(cb + 1) * 128],
                            ident[:], is_transpose=True, start=True, stop=True,
                        )
                        if cb % 2 == 0:
                            nc.scalar.copy(x1nT[cb][:, k * 128:(k + 1) * 128], tp[:])
                        else:
                            nc.vector.tensor_copy(
                                x1nT[cb][:, k * 128:(k + 1) * 128], tp[:])

                def mlp(col0, w):
                    """h1T[:, col0:col0+w] = gelu(W1^T x1nT + b1)."""
                    for nb in range(NF):
                        ps = psA.tile([128, 512], f32, tag="psA", name="psh")
                        for cb in range(NC):
                            nc.tensor.matmul(
                                ps[:, 0:w],
                                w1[cb][:, nb * 128:(nb + 1) * 128],
                                x1nT[cb][:, col0:col0 + w],
                                start=(cb == 0), stop=(cb == NC - 1),
                            )
                        nc.scalar.activation(
                            h1T[nb][:, col0:col0 + w], ps[:, 0:w],
                            AF.Gelu, bias=b1s[:, nb:nb + 1],
                        )

                def outproj(k):
                    o_sb = outp.tile([128, C], f32, tag="o", name="o_sb")
                    for gg in range(2):
                        ps = psA.tile([128, 512], f32, tag="psA", name="pso")
                        for nb in range(NF):
                            nc.tensor.matmul(
                                ps[:, 0:384],
                                h1T[nb][:, k * 128:(k + 1) * 128],
                                w2[nb][:, gg * 384:(gg + 1) * 384],
                                start=(nb == 0), stop=False,
                            )
                        nc.tensor.matmul(
                            ps[:, 0:384], ones_col[:],
                            b2s[:, gg * 384:(gg + 1) * 384],
                            start=False, stop=True,
                        )
                        nc.vector.tensor_add(
                            o_sb[:, gg * 384:(gg + 1) * 384], ps[:, 0:384],
                            x1[k][:, gg * 384:(gg + 1) * 384],
                        )
                    nc.sync.dma_start(out_d[k * 128:(k + 1) * 128, :], o_sb[:])

                for k in range(3):
                    strip(k, ccA_o, k * 128)
                mlp(0, 384)
                for k in range(3):
                    outproj(k)
                strip(3, ccB_o, 0)
                mlp(384, 128)
                outproj(3)

    nc.compile()
    return nc


def _prep_inputs(inputs):
    import ml_dtypes

    f = np.float32
    bf = ml_dtypes.bfloat16
    g1 = np.asarray(inputs["ln1_g"], f)
    b1r = np.asarray(inputs["ln1_b"], f)
    g2 = np.asarray(inputs["ln2_g"], f)
    b2r = np.asarray(inputs["ln2_b"], f)
    Wq, Wk, Wv = (np.asarray(inputs[k], f) for k in ("Wq", "Wk", "Wv"))
    W1, W2 = np.asarray(inputs["W1"], f), np.asarray(inputs["W2"], f)
    x = np.asarray(inputs["x"], f)

    def c(a, dtype=bf):
        return np.ascontiguousarray(a.astype(dtype))

    wq_f = g1[:, None] * Wq
    wk_f = g1[:, None] * Wk
    wv_f = g1[:, None] * Wv
    bq_f = b1r @ Wq + np.asarray(inputs["bq"], f)
    bk_f = b1r @ Wk + np.asarray(inputs["bk"], f)
    bv_f = b1r @ Wv + np.asarray(inputs["bv"], f)
    b1_f = b2r @ W1 + np.asarray(inputs["b1"], f)

    shared = {
        "w1": c(g2[:, None] * W1),
        "b1": np.ascontiguousarray(b1_f.reshape(NF, 128).T).astype(f),
        "w2": c(W2),
        "b2": c(np.asarray(inputs["b2"], f)[None, :]),
        "g1": c(g1[None, :]),
        "b1r": c(b1r[None, :]),
        "negtri": np.ascontiguousarray(
            NEG * np.tril(np.ones((128, 128), f), -1)).astype(f),
        "ident": c(np.eye(128, dtype=f)),
    }

    def bias2(b):
        out = np.zeros((128, 2), f)
        out[:, 0] = b[0:128]
        out[0:64, 1] = b[128:192]
        return out

    in_maps = []
    for core in range(8):
        b, g = core // 4, core % 4
        cols = slice(192 * g, 192 * (g + 1))
        msel = np.zeros((128, 4), f)
        msel[:, g] = 1.0
        selid = np.zeros((128, 512), f)
        selid[:, 128 * g:128 * (g + 1)] = np.eye(128, dtype=f)
        m = dict(shared)
        m["x"] = c(x[b])
        m["wq"] = c(wq_f[:, cols])
        m["wk"] = c(wk_f[:, cols])
        m["wv"] = c(wv_f[:, cols])
        m["bq"] = bias2(bq_f[cols])
        m["bk"] = bias2(bk_f[cols])
        m["bv"] = c(bv_f[cols][None, :])
        m["msel"] = msel
        m["selid"] = c(selid)
        in_maps.append(m)
    return in_maps


def kernel(**inputs):
    from concourse.bass_utils import run_bass_kernel_spmd

    if "nc" not in _CACHE:
        _CACHE["nc"] = _build_program()
    nc = _CACHE["nc"]

    in_maps = _prep_inputs(inputs)

    trace = bool(int(os.environ.get("KERNEL_TRACE", "0")))
    try:
        res = run_bass_kernel_spmd(nc, in_maps, core_ids=list(range(8)), trace=trace)
    except ModuleNotFoundError:
        res = run_bass_kernel_spmd(nc, in_maps, core_ids=list(range(8)), trace=False)
    _CACHE["last_result"] = res

    out = np.empty((B, T, C), np.float32)
    for core in range(8):
        b, g = core // 4, core % 4
        r = res.results[core]["out"]
        for k in range(4):
            out[b, 512 * k + 128 * g:512 * k + 128 * g + 128] = \
                r[128 * k:128 * (k + 1)]
    return out


# revision 13
# speedup vs baseline: 1.3423x; 1.0879x over previous
"""Trainium2 Bass kernel for a GPT-style decoder block (B=2, T=2048, C=768, H=12).

Sharding v2: 8 cores = 2 batches x 4 head-groups. Core (b, g) runs attention
for heads {3g, 3g+1, 3g+2} over ALL T rows (full causal triangle), then the
cores of a batch exchange attention outputs with two bf16 ReduceScatters so
each core runs LN2+MLP on a disjoint block-cyclic quarter of the rows
(strips rows [512*ci + 128*g, +128) for ci in 0..3).

The SPMD program is identical on every core; all core-dependence lives in the
inputs: sliced QKV weights, plus one-hot select tensors (msel / selident) that
route data into the right ReduceScatter slot and select the core's own rows
out of replicated LN1 activations.

Attention layout: scoresT[j, i] = k_i . q_j (reference computes K @ Q^T), so
the moving dim of the score matmuls is i (own-output tokens) and softmax sums
over partitions j via a ones-column that rides in V. P@V runs in fp8e4 with
MatmulPerfMode.DoubleRow (2x PE throughput); everything else is bf16 with
fp32 accumulation. Causality: score matmuls only cover i >= j-block start;
the diagonal 128-block gets -1e30 added pre-exp; sub-diagonal exp slots are
memset to 0.

LN gamma/beta are folded into the adjacent matmul weights/biases host-side.
"""

import os

import numpy as np

B, T, C = 2, 2048, 768
H, DH = 12, 64
HL = 3               # heads per core
F = 4 * C
NT = T // 128        # 16 token tiles
NC = C // 128        # 6 channel chunks
NF = F // 128        # 24 hidden chunks
EPS = 1e-3
NEG = -1e30

_CACHE = {}


def _build_program():
    import concourse.bass as bass  # noqa: F401
    import concourse.mybir as mybir
    import concourse.tile as tile
    from concourse import bacc

    dt = mybir.dt
    f32 = dt.float32
    bf16 = dt.bfloat16
    f8 = dt.float8e4
    AF = mybir.ActivationFunctionType
    ALU = mybir.AluOpType
    DR = mybir.MatmulPerfMode.DoubleRow

    nc = bacc.Bacc("TRN2", target_bir_lowering=False, debug=False, num_devices=8)

    # ---- DRAM I/O ----
    x_d = nc.dram_tensor("x", [T, C], bf16, kind="ExternalInput")
    wq_d = nc.dram_tensor("wq", [C, 128], bf16, kind="ExternalInput")
    wk_d = nc.dram_tensor("wk", [C, 128], bf16, kind="ExternalInput")
    wqkc_d = nc.dram_tensor("wqkc", [C, 128], bf16, kind="ExternalInput")
    bqkc_d = nc.dram_tensor("bqkc", [128, 1], f32, kind="ExternalInput")
    wv_d = nc.dram_tensor("wv", [C, HL * DH], bf16, kind="ExternalInput")
    bq_d = nc.dram_tensor("bq", [128, 2], f32, kind="ExternalInput")
    bk_d = nc.dram_tensor("bk", [128, 2], f32, kind="ExternalInput")
    bv_d = nc.dram_tensor("bv", [1, HL * DH], bf16, kind="ExternalInput")
    w1_d = nc.dram_tensor("w1", [C, F], bf16, kind="ExternalInput")
    b1_d = nc.dram_tensor("b1", [128, NF], f32, kind="ExternalInput")
    w2_d = nc.dram_tensor("w2", [F, C], bf16, kind="ExternalInput")
    b2_d = nc.dram_tensor("b2", [1, C], bf16, kind="ExternalInput")
    g1_d = nc.dram_tensor("g1", [1, C], bf16, kind="ExternalInput")
    b1r_d = nc.dram_tensor("b1r", [1, C], bf16, kind="ExternalInput")
    negtri_d = nc.dram_tensor("negtri", [128, 128], f32, kind="ExternalInput")
    ident_d = nc.dram_tensor("ident", [128, 128], bf16, kind="ExternalInput")
    msel_d = nc.dram_tensor("msel", [128, 4], f32, kind="ExternalInput")
    selid_d = nc.dram_tensor("selid", [128, 512], bf16, kind="ExternalInput")
    out_d = nc.dram_tensor("out", [512, C], f32, kind="ExternalOutput")

    with tile.TileContext(nc) as tc:
        with (
            tc.tile_pool(name="const", bufs=1) as constp,
            tc.tile_pool(name="dram", bufs=1, space="DRAM") as dramp,
            tc.tile_pool(name="psA", bufs=2, space="PSUM") as psA,
            tc.tile_pool(name="psPV", bufs=3, space="PSUM") as psPV,
            tc.tile_pool(name="psTr", bufs=2, space="PSUM") as psTr,
        ):
            # ---- constants ----
            negtri = constp.tile([128, 128], f32)
            nc.sync.dma_start(negtri[:], negtri_d[:])
            ident = constp.tile([128, 128], bf16)
            nc.sync.dma_start(ident[:], ident_d[:])
            msel = constp.tile([128, 4], f32)
            nc.sync.dma_start(msel[:], msel_d[:])
            selid = constp.tile([128, 512], bf16)
            nc.sync.dma_start(selid[:], selid_d[:])
            bqs = constp.tile([128, 2], f32)
            nc.sync.dma_start(bqs[:], bq_d[:])
            bks = constp.tile([128, 2], f32)
            nc.sync.dma_start(bks[:], bk_d[:])
            bqkcs = constp.tile([128, 1], f32)
            nc.sync.dma_start(bqkcs[:], bqkc_d[:])
            b1s = constp.tile([128, NF], f32)
            nc.sync.dma_start(b1s[:], b1_d[:])
            ones_col = constp.tile([1, 128], bf16)
            nc.vector.memset(ones_col[:], 1.0)
            eps_t = constp.tile([128, 1], f32)
            nc.vector.memset(eps_t[:], EPS)

            # broadcast ln1 gamma/beta rows to [128, C] via rank-1 matmuls
            g1s = constp.tile([1, C], bf16)
            nc.sync.dma_start(g1s[:], g1_d[:])
            b1rs = constp.tile([1, C], bf16)
            nc.sync.dma_start(b1rs[:], b1r_d[:])
            g1b = constp.tile([128, C], f32)
            b1rb = constp.tile([128, C], f32)
            for dst, src in ((g1b, g1s), (b1rb, b1rs)):
                for gg in range(2):
                    ps = psA.tile([128, 512], f32, tag="psA", name="psbc")
                    nc.tensor.matmul(
                        ps[:, 0:384], ones_col[:], src[:, gg * 384:(gg + 1) * 384],
                        start=True, stop=True,
                    )
                    nc.vector.tensor_copy(dst[:, gg * 384:(gg + 1) * 384], ps[:, 0:384])

            # persistent activations
            xn_bf = [constp.tile([128, C], bf16, name=f"xnbf{tb}") for tb in range(NT)]

            with (
                tc.tile_pool(name="QK", bufs=1) as qkp,
                tc.tile_pool(name="V8", bufs=1) as v8p,
            ):
                QT_ab = qkp.tile([128, T], bf16, name="QT_ab")
                QT_c = qkp.tile([64, T], bf16, name="QT_c")
                KT_ab = qkp.tile([128, T], bf16, name="KT_ab")
                KT_c = qkp.tile([64, T], bf16, name="KT_c")
                Vbf = [v8p.tile([128, HL, DH + 1], bf16, name=f"V_{tb}")
                       for tb in range(NT)]

                with tc.tile_pool(name="xnT", bufs=1) as xnTp:
                    xnT = [xnTp.tile([128, T], bf16, name=f"xnT{cb}") for cb in range(NC)]

                    # ===== Phase A: LN1 over all T + transpose to xnT =====
                    with (
                        tc.tile_pool(name="xin", bufs=3) as xinp,
                        tc.tile_pool(name="stat", bufs=4) as statp,
                    ):
                        for tb in range(NT):
                            xt = xinp.tile([128, C], bf16, tag="xt", name="xt")
                            nc.sync.dma_start(xt[:], x_d[tb * 128:(tb + 1) * 128, :])
                            st6 = statp.tile([128, 2, 6], f32, tag="st6", name="st6")
                            for gg in range(2):
                                nc.vector.bn_stats(
                                    st6[:, gg, :], xt[:, gg * 384:(gg + 1) * 384]
                                )
                            st2 = statp.tile([128, 2], f32, tag="st2", name="st2")
                            nc.vector.bn_aggr(st2[:], st6[:])
                            std = statp.tile([128, 1], f32, tag="std", name="std")
                            nc.scalar.activation(std[:], st2[:, 1:2], AF.Sqrt, bias=eps_t[:])
                            rstd = statp.tile([128, 1], f32, tag="rstd", name="rstd")
                            nc.vector.reciprocal(rstd[:], std[:])
                            nmb = statp.tile([128, 1], f32, tag="nmb", name="nmb")
                            nc.vector.tensor_scalar(
                                nmb[:], st2[:, 0:1], rstd[:], -1.0,
                                op0=ALU.mult, op1=ALU.mult,
                            )
                            nc.scalar.activation(
                                xn_bf[tb][:], xt[:], AF.Identity,
                                bias=nmb[:], scale=rstd[:],
                            )
                            for cb in range(NC):
                                tp = psTr.tile([128, 128], bf16, tag="psTr", name="tp")
                                nc.tensor.matmul(
                                    tp[:], xn_bf[tb][:, cb * 128:(cb + 1) * 128],
                                    ident[:], is_transpose=True, start=True, stop=True,
                                )
                                if cb % 2 == 0:
                                    nc.scalar.copy(
                                        xnT[cb][:, tb * 128:(tb + 1) * 128], tp[:])
                                else:
                                    nc.vector.tensor_copy(
                                        xnT[cb][:, tb * 128:(tb + 1) * 128], tp[:])

                    # ===== Phase B: QKV projections (3 heads) =====
                    with tc.tile_pool(name="wqkv", bufs=1) as wp:
                        wq = [wp.tile([128, 128], bf16, name=f"wq{cb}") for cb in range(NC)]
                        wk = [wp.tile([128, 128], bf16, name=f"wk{cb}") for cb in range(NC)]
                        wqkc = [wp.tile([128, 128], bf16, name=f"wqkc{cb}") for cb in range(NC)]
                        wv = [wp.tile([128, HL * DH], bf16, name=f"wv{cb}") for cb in range(NC)]
                        for cb in range(NC):
                            nc.sync.dma_start(wq[cb][:], wq_d[cb * 128:(cb + 1) * 128, :])
                            nc.sync.dma_start(wk[cb][:], wk_d[cb * 128:(cb + 1) * 128, :])
                            nc.sync.dma_start(wqkc[cb][:], wqkc_d[cb * 128:(cb + 1) * 128, :])
                            nc.sync.dma_start(wv[cb][:], wv_d[cb * 128:(cb + 1) * 128, :])
                        bvs = wp.tile([1, HL * DH], bf16, name="bvs")
                        nc.sync.dma_start(bvs[:], bv_d[:])

                        for jb in range(T // 512):
                            sl = slice(jb * 512, (jb + 1) * 512)
                            for (W, dst_ab, bias) in (
                                (wq, QT_ab, bqs),
                                (wk, KT_ab, bks),
                            ):
                                ps = psA.tile([128, 512], f32, tag="psA", name="psq")
                                for cb in range(NC):
                                    nc.tensor.matmul(
                                        ps[:], W[cb][:], xnT[cb][:, sl],
                                        start=(cb == 0), stop=(cb == NC - 1),
                                    )
                                nc.vector.tensor_scalar_add(
                                    dst_ab[:, sl], ps[:], bias[:, 0:1])
                            # packed third-head Q (rows 0:64) and K (rows 64:128)
                            ps2 = psA.tile([128, 512], f32, tag="psA", name="psq2")
                            for cb in range(NC):
                                nc.tensor.matmul(
                                    ps2[:], wqkc[cb][:], xnT[cb][:, sl],
                                    start=(cb == 0), stop=(cb == NC - 1),
                                )
                            nc.vector.tensor_scalar_add(
                                QT_c[:, sl], ps2[0:64, :], bqkcs[0:64, 0:1])
                            nc.vector.tensor_scalar_add(
                                KT_c[:, sl], ps2[64:128, :], bqkcs[64:128, 0:1])

                        for tb in range(NT):
                            nc.vector.memset(Vbf[tb][:, :, DH:DH + 1], 1.0)
                            psv = psA.tile([128, 512], f32, tag="psA", name="psv")
                            for cb in range(NC):
                                nc.tensor.matmul(
                                    psv[:, 0:HL * DH],
                                    xnT[cb][:, tb * 128:(tb + 1) * 128],
                                    wv[cb][:],
                                    start=(cb == 0), stop=False,
                                )
                            nc.tensor.matmul(
                                psv[:, 0:HL * DH], ones_col[:], bvs[:],
                                start=False, stop=True,
                            )
                            nc.scalar.copy(
                                Vbf[tb][:, :, 0:DH],
                                psv[:, 0:HL * DH].rearrange(
                                    "p (h d) -> p h d", d=DH),
                            )

                # ===== Phase C: attention (3 heads, full causal) =====
                qslices = (
                    (QT_ab, 0), (QT_ab, 64), (QT_c, 0),
                )
                kslices = (
                    (KT_ab, 0), (KT_ab, 64), (KT_c, 0),
                )

                with (
                    tc.tile_pool(name="exps", bufs=1) as expp,
                    tc.tile_pool(name="ysb", bufs=8) as ysbp,
                    tc.tile_pool(name="pad", bufs=4) as padp,
                    tc.tile_pool(name="yT", bufs=3) as yTp,
                ):
                    # DRAM bounce buffers for the three ReduceScatters
                    cc1_i = dramp.tile([1024, 4 * HL * (DH + 1)], bf16, name="cc1_i")
                    cc1_o = dramp.tile([256, 4 * HL * (DH + 1)], bf16, name="cc1_o")
                    cc2_i = dramp.tile([512, 4 * HL * (DH + 1)], bf16, name="cc2_i")
                    cc2_o = dramp.tile([128, 4 * HL * (DH + 1)], bf16, name="cc2_o")
                    cc3_i = dramp.tile([512, 4 * HL * (DH + 1)], bf16, name="cc3_i")
                    cc3_o = dramp.tile([128, 4 * HL * (DH + 1)], bf16, name="cc3_o")

                    # expST tiles allocated per (ci, h) below
                    for ci in range(4):
                        i0 = ci * 512
                        nj = 4 * (ci + 1)
                        exps = {}
                        for h in range(HL):
                            qt, qoff = qslices[h]
                            kt, koff = kslices[h]
                            for jb in range(nj):
                                et = expp.tile([128, 512], bf16, tag=f"e{h}_{jb}",
                                               name=f"e{h}_{jb}")
                                exps[(h, jb)] = et
                                r = jb - 4 * ci
                                off = 128 * r if r >= 0 else 0
                                w = 512 - off
                                pss = psA.tile([128, 512], f32, tag="psA",
                                               name="pss")
                                nc.tensor.matmul(
                                    pss[:, 0:w],
                                    qt[qoff:qoff + 64, jb * 128:(jb + 1) * 128],
                                    kt[koff:koff + 64, i0 + off:i0 + 512],
                                    start=True, stop=True,
                                )
                                if r >= 0:
                                    nc.vector.tensor_add(
                                        pss[:, 0:128], pss[:, 0:128], negtri[:])
                                    if off:
                                        nc.vector.memset(et[:, 0:off], 0.0)
                                nc.scalar.activation(
                                    et[:, off:512], pss[:, 0:w],
                                    AF.Exp, scale=0.125,
                                )
                        # P@V per head (bf16, accumulate over j-blocks)
                        psys = []
                        for h in range(HL):
                            psy = psPV.tile([128, 512], f32, tag="psPV", name="psy")
                            psys.append(psy)
                            for jb in range(nj):
                                nc.tensor.matmul(
                                    psy[0:DH + 1, :],
                                    Vbf[jb][:, h, :],
                                    exps[(h, jb)][:],
                                    start=(jb == 0), stop=(jb == nj - 1),
                                )
                        # transpose yT -> token-major y_sb tiles
                        y_sb = [ysbp.tile([128, HL * (DH + 1)], bf16, tag="ysb",
                                          name=f"ysb{ib}") for ib in range(4)]
                        for h in range(HL):
                            yT_bf = yTp.tile([128, 512], bf16, tag="yT", name="yT")
                            nc.vector.tensor_copy(yT_bf[0:DH + 1, :],
                                                  psys[h][0:DH + 1, :])
                            for ib in range(4):
                                tp = psTr.tile([128, 128], bf16, tag="psTr",
                                               name="tpy")
                                nc.tensor.matmul(
                                    tp[:], yT_bf[:, ib * 128:(ib + 1) * 128],
                                    ident[:], is_transpose=True,
                                    start=True, stop=True,
                                )
                                nc.vector.tensor_copy(
                                    y_sb[ib][:, h * (DH + 1):(h + 1) * (DH + 1)],
                                    tp[:, 0:DH + 1])
                        # pad into 4 receiver slots and ship to the RS buffer
                        W780 = 4 * HL * (DH + 1)
                        for ib in range(4):
                            pad = padp.tile([128, 4, HL * (DH + 1)], bf16, tag="pad",
                                            name="pad")
                            for t in range(4):
                                nc.vector.tensor_scalar_mul(
                                    pad[:, t, :], y_sb[ib][:], msel[:, t:t + 1])
                            if ci < 2:
                                nc.gpsimd.dma_start(
                                    cc1_i[ib * 256 + ci * 128:
                                          ib * 256 + ci * 128 + 128, :],
                                    pad[:].rearrange("p t d -> p (t d)"),
                                )
                            elif ci == 2:
                                nc.gpsimd.dma_start(
                                    cc2_i[ib * 128:(ib + 1) * 128, :],
                                    pad[:].rearrange("p t d -> p (t d)"),
                                )
                            else:
                                nc.gpsimd.dma_start(
                                    cc3_i[ib * 128:(ib + 1) * 128, :],
                                    pad[:].rearrange("p t d -> p (t d)"),
                                )
                        if ci >= 1:
                            cci, cco = [(cc1_i, cc1_o), (cc2_i, cc2_o),
                                        (cc3_i, cc3_o)][ci - 1]
                            nc.gpsimd.collective_compute(
                                "ReduceScatter",
                                mybir.AluOpType.add,
                                replica_groups=[[0, 1, 2, 3], [4, 5, 6, 7]],
                                ins=[cci[:]],
                                outs=[cco[:]],
                            )

            # ===== Phase D/E: readback, residual, LN2, MLP =====
            W65 = DH + 1
            W195 = HL * W65
            with (
                tc.tile_pool(name="yin", bufs=2) as yinp,
                tc.tile_pool(name="x1p", bufs=1) as x1p,
                tc.tile_pool(name="x1nT", bufs=1) as x1nTp,
                tc.tile_pool(name="w1p", bufs=1) as w1p,
                tc.tile_pool(name="w2p", bufs=1) as w2p,
                tc.tile_pool(name="h1T", bufs=1) as h1Tp,
                tc.tile_pool(name="stat2", bufs=4) as stat2p,
                tc.tile_pool(name="dtmp", bufs=4) as dtmpp,
                tc.tile_pool(name="outp", bufs=2) as outp,
            ):
                w1 = [w1p.tile([128, F], bf16, name=f"w1_{cb}") for cb in range(NC)]
                for cb in range(NC):
                    nc.sync.dma_start(w1[cb][:], w1_d[cb * 128:(cb + 1) * 128, :])
                w2 = [w2p.tile([128, C], bf16, name=f"w2_{nb}") for nb in range(NF)]
                for nb in range(NF):
                    nc.sync.dma_start(w2[nb][:], w2_d[nb * 128:(nb + 1) * 128, :])
                b2s = w2p.tile([1, C], bf16, name="b2s")
                nc.sync.dma_start(b2s[:], b2_d[:])

                x1 = [x1p.tile([128, C], f32, name=f"x1_{k}") for k in range(4)]
                x1nT = [x1nTp.tile([128, 512], bf16, name=f"x1nT{cb}")
                        for cb in range(NC)]
                h1T = [h1Tp.tile([128, 512], bf16, name=f"h1T{nb}")
                       for nb in range(NF)]

                def strip(k, cc_o, row0):
                    """x1[k] = sel(xn)*g1 + b1r + y/denom; LN2 -> x1nT cols."""
                    yall = yinp.tile([128, 4 * W195], bf16, tag="yin", name="yin")
                    nc.sync.dma_start(yall[:], cc_o[row0:row0 + 128, :])
                    yv = yall[:].rearrange("p (q d) -> p q d", d=W65)
                    rec = dtmpp.tile([128, H], f32, tag="rec", name="rec")
                    nc.vector.tensor_copy(rec[:], yv[:, :, DH])
                    nc.vector.reciprocal(rec[:], rec[:])
                    yf = dtmpp.tile([128, C], f32, tag="yf", name="yf")
                    for hh in range(H):
                        nc.vector.tensor_scalar_mul(
                            yf[:, hh * DH:(hh + 1) * DH],
                            yv[:, hh, 0:DH],
                            rec[:, hh:hh + 1],
                        )
                    # select own xn rows via one-hot identity matmul
                    for gg in range(2):
                        psx = psA.tile([128, 512], f32, tag="psA", name="psx")
                        for t in range(4):
                            nc.tensor.matmul(
                                psx[:, 0:384],
                                selid[:, t * 128:(t + 1) * 128],
                                xn_bf[4 * k + t][:, gg * 384:(gg + 1) * 384],
                                start=(t == 0), stop=(t == 3),
                            )
                        gs = slice(gg * 384, (gg + 1) * 384)
                        nc.vector.tensor_tensor(
                            x1[k][:, gs], psx[:, 0:384], g1b[:, gs], ALU.mult)
                        nc.vector.tensor_add(x1[k][:, gs], x1[k][:, gs], b1rb[:, gs])
                        nc.vector.tensor_add(x1[k][:, gs], x1[k][:, gs], yf[:, gs])
                    # LN2
                    st6 = stat2p.tile([128, 2, 6], f32, tag="st6", name="st6b")
                    for gg in range(2):
                        nc.vector.bn_stats(
                            st6[:, gg, :], x1[k][:, gg * 384:(gg + 1) * 384])
                    st2 = stat2p.tile([128, 2], f32, tag="st2", name="st2b")
                    nc.vector.bn_aggr(st2[:], st6[:])
                    std = stat2p.tile([128, 1], f32, tag="std", name="stdb")
                    nc.scalar.activation(std[:], st2[:, 1:2], AF.Sqrt, bias=eps_t[:])
                    rstd = stat2p.tile([128, 1], f32, tag="rstd", name="rstdb")
                    nc.vector.reciprocal(rstd[:], std[:])
                    nmb = stat2p.tile([128, 1], f32, tag="nmb", name="nmbb")
                    nc.vector.tensor_scalar(
                        nmb[:], st2[:, 0:1], rstd[:], -1.0,
                        op0=ALU.mult, op1=ALU.mult,
                    )
                    x1n = dtmpp.tile([128, C], bf16, tag="x1n", name="x1n")
                    nc.scalar.activation(
                        x1n[:], x1[k][:], AF.Identity, bias=nmb[:], scale=rstd[:])
                    for cb in range(NC):
                        tp = psTr.tile([128, 128], bf16, tag="psTr", name="tpb")
                        nc.tensor.matmul(
                            tp[:], x1n[:, cb * 128:(cb + 1) * 128],
                            ident[:], is_transpose=True, start=True, stop=True,
                        )
                        if cb % 2 == 0:
                            nc.scalar.copy(x1nT[cb][:, k * 128:(k + 1) * 128], tp[:])
                        else:
                            nc.vector.tensor_copy(
                                x1nT[cb][:, k * 128:(k + 1) * 128], tp[:])

                def mlp(col0, w):
                    """h1T[:, col0:col0+w] = gelu(W1^T x1nT + b1)."""
                    for nb in range(NF):
                        ps = psA.tile([128, 512], f32, tag="psA", name="psh")
                        for cb in range(NC):
                            nc.tensor.matmul(
                                ps[:, 0:w],
                                w1[cb][:, nb * 128:(nb + 1) * 128],
                                x1nT[cb][:, col0:col0 + w],
                                start=(cb == 0), stop=(cb == NC - 1),
                            )
                        nc.scalar.activation(
                            h1T[nb][:, col0:col0 + w], ps[:, 0:w],
                            AF.Gelu, bias=b1s[:, nb:nb + 1],
                        )

                def outproj(k):
                    o_sb = outp.tile([128, C], f32, tag="o", name="o_sb")
                    for gg in range(2):
                        ps = psA.tile([128, 512], f32, tag="psA", name="pso")
                        for nb in range(NF):
                            nc.tensor.matmul(
                                ps[:, 0:384],
                                h1T[nb][:, k * 128:(k + 1) * 128],
                                w2[nb][:, gg * 384:(gg + 1) * 384],
                                start=(nb == 0), stop=False,
                            )
                        nc.tensor.matmul(
                            ps[:, 0:384], ones_col[:],
                            b2s[:, gg * 384:(gg + 1) * 384],
                            start=False, stop=True,
                        )
                        nc.vector.tensor_add(
                            o_sb[:, gg * 384:(gg + 1) * 384], ps[:, 0:384],
                            x1[k][:, gg * 384:(gg + 1) * 384],
                        )
                    nc.sync.dma_start(out_d[k * 128:(k + 1) * 128, :], o_sb[:])

                strip(0, cc1_o, 0)
                strip(1, cc1_o, 128)
                strip(2, cc2_o, 0)
                mlp(0, 384)
                for k in range(3):
                    outproj(k)
                strip(3, cc3_o, 0)
                mlp(384, 128)
                outproj(3)

    nc.compile()
    return nc


def _prep_inputs(inputs):
    import ml_dtypes

    f = np.float32
    bf = ml_dtypes.bfloat16
    g1 = np.asarray(inputs["ln1_g"], f)
    b1r = np.asarray(inputs["ln1_b"], f)
    g2 = np.asarray(inputs["ln2_g"], f)
    b2r = np.asarray(inputs["ln2_b"], f)
    Wq, Wk, Wv = (np.asarray(inputs[k], f) for k in ("Wq", "Wk", "Wv"))
    W1, W2 = np.asarray(inputs["W1"], f), np.asarray(inputs["W2"], f)
    x = np.asarray(inputs["x"], f)

    def c(a, dtype=bf):
        return np.ascontiguousarray(a.astype(dtype))

    wq_f = g1[:, None] * Wq
    wk_f = g1[:, None] * Wk
    wv_f = g1[:, None] * Wv
    bq_f = b1r @ Wq + np.asarray(inputs["bq"], f)
    bk_f = b1r @ Wk + np.asarray(inputs["bk"], f)
    bv_f = b1r @ Wv + np.asarray(inputs["bv"], f)
    b1_f = b2r @ W1 + np.asarray(inputs["b1"], f)

    shared = {
        "w1": c(g2[:, None] * W1),
        "b1": np.ascontiguousarray(b1_f.reshape(NF, 128).T).astype(f),
        "w2": c(W2),
        "b2": c(np.asarray(inputs["b2"], f)[None, :]),
        "g1": c(g1[None, :]),
        "b1r": c(b1r[None, :]),
        "negtri": np.ascontiguousarray(
            NEG * np.tril(np.ones((128, 128), f), -1)).astype(f),
        "ident": c(np.eye(128, dtype=f)),
    }

    def bias2(b):
        out = np.zeros((128, 2), f)
        out[:, 0] = b[0:128]
        out[0:64, 1] = b[128:192]
        return out

    in_maps = []
    for core in range(8):
        b, g = core // 4, core % 4
        cols = slice(192 * g, 192 * (g + 1))
        msel = np.zeros((128, 4), f)
        msel[:, g] = 1.0
        selid = np.zeros((128, 512), f)
        selid[:, 128 * g:128 * (g + 1)] = np.eye(128, dtype=f)
        wq_g = wq_f[:, cols]
        wk_g = wk_f[:, cols]
        bq_g = bq_f[cols]
        bk_g = bk_f[cols]
        m = dict(shared)
        m["x"] = c(x[b])
        m["wq"] = c(wq_g[:, 0:128])
        m["wk"] = c(wk_g[:, 0:128])
        m["wqkc"] = c(np.concatenate([wq_g[:, 128:192], wk_g[:, 128:192]], axis=1))
        m["bqkc"] = np.ascontiguousarray(
            np.concatenate([bq_g[128:192], bk_g[128:192]])[:, None]).astype(f)
        m["wv"] = c(wv_f[:, cols])
        m["bq"] = bias2(bq_g)
        m["bk"] = bias2(bk_g)
        m["bv"] = c(bv_f[cols][None, :])
        m["msel"] = msel
        m["selid"] = c(selid)
        in_maps.append(m)
    return in_maps


def kernel(**inputs):
    from concourse.bass_utils import run_bass_kernel_spmd

    if "nc" not in _CACHE:
        _CACHE["nc"] = _build_program()
    nc = _CACHE["nc"]

    in_maps = _prep_inputs(inputs)

    trace = bool(int(os.environ.get("KERNEL_TRACE", "0")))
    try:
        res = run_bass_kernel_spmd(nc, in_maps, core_ids=list(range(8)), trace=trace)
    except ModuleNotFoundError:
        res = run_bass_kernel_spmd(nc, in_maps, core_ids=list(range(8)), trace=False)
    _CACHE["last_result"] = res

    out = np.empty((B, T, C), np.float32)
    for core in range(8):
        b, g = core // 4, core % 4
        r = res.results[core]["out"]
        for k in range(4):
            out[b, 512 * k + 128 * g:512 * k + 128 * g + 128] = \
                r[128 * k:128 * (k + 1)]
    return out


# revision 19
# speedup vs baseline: 1.3947x; 1.0390x over previous
"""Trainium2 Bass kernel for a GPT-style decoder block (B=2, T=2048, C=768, H=12).

Sharding v2: 8 cores = 2 batches x 4 head-groups. Core (b, g) runs attention
for heads {3g, 3g+1, 3g+2} over ALL T rows (full causal triangle), then the
cores of a batch exchange attention outputs with two bf16 ReduceScatters so
each core runs LN2+MLP on a disjoint block-cyclic quarter of the rows
(strips rows [512*ci + 128*g, +128) for ci in 0..3).

The SPMD program is identical on every core; all core-dependence lives in the
inputs: sliced QKV weights, plus one-hot select tensors (msel / selident) that
route data into the right ReduceScatter slot and select the core's own rows
out of replicated LN1 activations.

Attention layout: scoresT[j, i] = k_i . q_j (reference computes K @ Q^T), so
the moving dim of the score matmuls is i (own-output tokens) and softmax sums
over partitions j via a ones-column that rides in V. P@V runs in fp8e4 with
MatmulPerfMode.DoubleRow (2x PE throughput); everything else is bf16 with
fp32 accumulation. Causality: score matmuls only cover i >= j-block start;
the diagonal 128-block gets -1e30 added pre-exp; sub-diagonal exp slots are
memset to 0.

LN gamma/beta are folded into the adjacent matmul weights/biases host-side.
"""

import os

import numpy as np

B, T, C = 2, 2048, 768
H, DH = 12, 64
HL = 3               # heads per core
F = 4 * C
NT = T // 128        # 16 token tiles
NC = C // 128        # 6 channel chunks
NF = F // 128        # 24 hidden chunks
EPS = 1e-3
NEG = -1e30

_CACHE = {}


def _build_program():
    import concourse.bass as bass  # noqa: F401
    import concourse.mybir as mybir
    import concourse.tile as tile
    from concourse import bacc

    dt = mybir.dt
    f32 = dt.float32
    bf16 = dt.bfloat16
    f8 = dt.float8e4
    AF = mybir.ActivationFunctionType
    ALU = mybir.AluOpType
    DR = mybir.MatmulPerfMode.DoubleRow

    nc = bacc.Bacc("TRN2", target_bir_lowering=False, debug=False, num_devices=8)

    # ---- DRAM I/O ----
    x_d = nc.dram_tensor("x", [T, C], bf16, kind="ExternalInput")
    wq_d = nc.dram_tensor("wq", [C, 128], bf16, kind="ExternalInput")
    wk_d = nc.dram_tensor("wk", [C, 128], bf16, kind="ExternalInput")
    wqkc_d = nc.dram_tensor("wqkc", [C, 128], bf16, kind="ExternalInput")
    bqkc_d = nc.dram_tensor("bqkc", [128, 1], f32, kind="ExternalInput")
    wv_d = nc.dram_tensor("wv", [C, HL * DH], bf16, kind="ExternalInput")
    bq_d = nc.dram_tensor("bq", [128, 2], f32, kind="ExternalInput")
    bk_d = nc.dram_tensor("bk", [128, 2], f32, kind="ExternalInput")
    bv_d = nc.dram_tensor("bv", [1, HL * DH], bf16, kind="ExternalInput")
    w1_d = nc.dram_tensor("w1", [C, F], bf16, kind="ExternalInput")
    b1_d = nc.dram_tensor("b1", [128, NF], f32, kind="ExternalInput")
    w2_d = nc.dram_tensor("w2", [F, C], bf16, kind="ExternalInput")
    b2_d = nc.dram_tensor("b2", [1, C], bf16, kind="ExternalInput")
    g1_d = nc.dram_tensor("g1", [1, C], bf16, kind="ExternalInput")
    b1r_d = nc.dram_tensor("b1r", [1, C], bf16, kind="ExternalInput")
    negtri_d = nc.dram_tensor("negtri", [128, 128], f32, kind="ExternalInput")
    ident_d = nc.dram_tensor("ident", [128, 128], bf16, kind="ExternalInput")
    msel_d = nc.dram_tensor("msel", [128, 4], f32, kind="ExternalInput")
    selid_d = nc.dram_tensor("selid", [128, 512], bf16, kind="ExternalInput")
    out_d = nc.dram_tensor("out", [512, C], f32, kind="ExternalOutput")

    with tile.TileContext(nc) as tc:
        with (
            tc.tile_pool(name="const", bufs=1) as constp,
            tc.tile_pool(name="dram", bufs=1, space="DRAM") as dramp,
            tc.tile_pool(name="psA", bufs=2, space="PSUM") as psA,
            tc.tile_pool(name="psPV", bufs=3, space="PSUM") as psPV,
            tc.tile_pool(name="psDen", bufs=2, space="PSUM") as psDen,
            tc.tile_pool(name="psTr", bufs=1, space="PSUM") as psTr,
        ):
            # ---- constants ----
            negtri = constp.tile([128, 128], f32)
            nc.sync.dma_start(negtri[:], negtri_d[:])
            ident = constp.tile([128, 128], bf16)
            nc.sync.dma_start(ident[:], ident_d[:])
            msel = constp.tile([128, 4], f32)
            nc.sync.dma_start(msel[:], msel_d[:])
            selid = constp.tile([128, 512], bf16)
            nc.sync.dma_start(selid[:], selid_d[:])
            bqs = constp.tile([128, 2], f32)
            nc.sync.dma_start(bqs[:], bq_d[:])
            bks = constp.tile([128, 2], f32)
            nc.sync.dma_start(bks[:], bk_d[:])
            bqkcs = constp.tile([128, 1], f32)
            nc.sync.dma_start(bqkcs[:], bqkc_d[:])
            b1s = constp.tile([128, NF], f32)
            nc.sync.dma_start(b1s[:], b1_d[:])
            ones_col = constp.tile([1, 128], bf16)
            nc.vector.memset(ones_col[:], 1.0)
            eps_t = constp.tile([128, 1], f32)
            nc.vector.memset(eps_t[:], EPS)

            # broadcast ln1 gamma/beta rows to [128, C] via rank-1 matmuls
            g1s = constp.tile([1, C], bf16)
            nc.sync.dma_start(g1s[:], g1_d[:])
            b1rs = constp.tile([1, C], bf16)
            nc.sync.dma_start(b1rs[:], b1r_d[:])
            g1b = constp.tile([128, C], f32)
            b1rb = constp.tile([128, C], f32)
            for dst, src in ((g1b, g1s), (b1rb, b1rs)):
                for gg in range(2):
                    ps = psA.tile([128, 512], f32, tag="psA", name="psbc")
                    nc.tensor.matmul(
                        ps[:, 0:384], ones_col[:], src[:, gg * 384:(gg + 1) * 384],
                        start=True, stop=True,
                    )
                    nc.vector.tensor_copy(dst[:, gg * 384:(gg + 1) * 384], ps[:, 0:384])

            # persistent activations
            xn_bf = [constp.tile([128, C], bf16, name=f"xnbf{tb}") for tb in range(NT)]

            w1p = tc.alloc_tile_pool(name="w1p", bufs=1)
            w2p = tc.alloc_tile_pool(name="w2p", bufs=1)
            w1 = [w1p.tile([128, F], bf16, name=f"w1_{cb}") for cb in range(NC)]
            w2 = [w2p.tile([128, C], bf16, name=f"w2_{nb}") for nb in range(NF)]
            b2s = w2p.tile([1, C], bf16, name="b2s")
            with (
                tc.tile_pool(name="QK", bufs=1) as qkp,
                tc.tile_pool(name="V8", bufs=1) as v8p,
            ):
                QT_ab = qkp.tile([128, T], bf16, name="QT_ab")
                QT_c = qkp.tile([64, T], bf16, name="QT_c")
                KT_ab = qkp.tile([128, T], bf16, name="KT_ab")
                KT_c = qkp.tile([64, T], bf16, name="KT_c")
                V8 = [v8p.tile([128, 2, HL, DH], f8, name=f"V8_{jp}")
                      for jp in range(NT // 2)]
                ones8 = v8p.tile([128, 2, DH], f8, name="ones8")
                nc.vector.memset(ones8[:], 1.0)

                with tc.tile_pool(name="xnT", bufs=1) as xnTp:
                    xnT = [xnTp.tile([128, T], bf16, name=f"xnT{cb}") for cb in range(NC)]

                    # ===== Phase A: LN1 over all T + transpose to xnT =====
                    with (
                        tc.tile_pool(name="xin", bufs=3) as xinp,
                        tc.tile_pool(name="stat", bufs=4) as statp,
                    ):
                        for tb in range(NT):
                            xt = xinp.tile([128, C], bf16, tag="xt", name="xt")
                            nc.sync.dma_start(xt[:], x_d[tb * 128:(tb + 1) * 128, :])
                            st6 = statp.tile([128, 2, 6], f32, tag="st6", name="st6")
                            for gg in range(2):
                                nc.vector.bn_stats(
                                    st6[:, gg, :], xt[:, gg * 384:(gg + 1) * 384]
                                )
                            st2 = statp.tile([128, 2], f32, tag="st2", name="st2")
                            nc.vector.bn_aggr(st2[:], st6[:])
                            std = statp.tile([128, 1], f32, tag="std", name="std")
                            nc.scalar.activation(std[:], st2[:, 1:2], AF.Sqrt, bias=eps_t[:])
                            rstd = statp.tile([128, 1], f32, tag="rstd", name="rstd")
                            nc.vector.reciprocal(rstd[:], std[:])
                            nmb = statp.tile([128, 1], f32, tag="nmb", name="nmb")
                            nc.vector.tensor_scalar(
                                nmb[:], st2[:, 0:1], rstd[:], -1.0,
                                op0=ALU.mult, op1=ALU.mult,
                            )
                            nc.scalar.activation(
                                xn_bf[tb][:], xt[:], AF.Identity,
                                bias=nmb[:], scale=rstd[:],
                            )
                            for cb in range(NC):
                                tp = psTr.tile([128, 128], bf16, tag="psTr", name="tp")
                                nc.tensor.matmul(
                                    tp[:], xn_bf[tb][:, cb * 128:(cb + 1) * 128],
                                    ident[:], is_transpose=True, start=True, stop=True,
                                )
                                if cb % 2 == 0:
                                    nc.scalar.copy(
                                        xnT[cb][:, tb * 128:(tb + 1) * 128], tp[:])
                                else:
                                    nc.vector.tensor_copy(
                                        xnT[cb][:, tb * 128:(tb + 1) * 128], tp[:])

                    # ===== Phase B: QKV projections (3 heads) =====
                    with tc.tile_pool(name="wqkv", bufs=1) as wp:
                        wq = [wp.tile([128, 128], bf16, name=f"wq{cb}") for cb in range(NC)]
                        wk = [wp.tile([128, 128], bf16, name=f"wk{cb}") for cb in range(NC)]
                        wqkc = [wp.tile([128, 128], bf16, name=f"wqkc{cb}") for cb in range(NC)]
                        wv = [wp.tile([128, HL * DH], bf16, name=f"wv{cb}") for cb in range(NC)]
                        for cb in range(NC):
                            nc.sync.dma_start(wq[cb][:], wq_d[cb * 128:(cb + 1) * 128, :])
                            nc.sync.dma_start(wk[cb][:], wk_d[cb * 128:(cb + 1) * 128, :])
                            nc.sync.dma_start(wqkc[cb][:], wqkc_d[cb * 128:(cb + 1) * 128, :])
                            nc.sync.dma_start(wv[cb][:], wv_d[cb * 128:(cb + 1) * 128, :])
                        bvs = wp.tile([1, HL * DH], bf16, name="bvs")
                        nc.sync.dma_start(bvs[:], bv_d[:])
                        for cb in range(NC):
                            nc.sync.dma_start(w1[cb][:], w1_d[cb * 128:(cb + 1) * 128, :])
                        for nb in range(NF):
                            nc.sync.dma_start(w2[nb][:], w2_d[nb * 128:(nb + 1) * 128, :])
                        nc.sync.dma_start(b2s[:], b2_d[:])

                        for jb in range(T // 512):
                            sl = slice(jb * 512, (jb + 1) * 512)
                            for (W, dst_ab, bias) in (
                                (wq, QT_ab, bqs),
                                (wk, KT_ab, bks),
                            ):
                                ps = psA.tile([128, 512], f32, tag="psA", name="psq")
                                for cb in range(NC):
                                    nc.tensor.matmul(
                                        ps[:], W[cb][:], xnT[cb][:, sl],
                                        start=(cb == 0), stop=(cb == NC - 1),
                                    )
                                nc.vector.tensor_scalar_add(
                                    dst_ab[:, sl], ps[:], bias[:, 0:1])
                            # packed third-head Q (rows 0:64) and K (rows 64:128)
                            ps2 = psA.tile([128, 512], f32, tag="psA", name="psq2")
                            for cb in range(NC):
                                nc.tensor.matmul(
                                    ps2[:], wqkc[cb][:], xnT[cb][:, sl],
                                    start=(cb == 0), stop=(cb == NC - 1),
                                )
                            nc.vector.tensor_scalar_add(
                                QT_c[:, sl], ps2[0:64, :], bqkcs[0:64, 0:1])
                            nc.vector.tensor_scalar_add(
                                KT_c[:, sl], ps2[64:128, :], bqkcs[64:128, 0:1])

                        for tb in range(NT):
                            psv = psA.tile([128, 512], f32, tag="psA", name="psv")
                            for cb in range(NC):
                                nc.tensor.matmul(
                                    psv[:, 0:HL * DH],
                                    xnT[cb][:, tb * 128:(tb + 1) * 128],
                                    wv[cb][:],
                                    start=(cb == 0), stop=False,
                                )
                            nc.tensor.matmul(
                                psv[:, 0:HL * DH], ones_col[:], bvs[:],
                                start=False, stop=True,
                            )
                            nc.scalar.copy(
                                V8[tb // 2][:, tb % 2, :, :],
                                psv[:, 0:HL * DH].rearrange(
                                    "p (h d) -> p h d", d=DH),
                            )

                # ===== Phase C: attention (3 heads, full causal) =====
                qslices = (
                    (QT_ab, 0), (QT_ab, 64), (QT_c, 0),
                )
                kslices = (
                    (KT_ab, 0), (KT_ab, 64), (KT_c, 0),
                )

                with (
                    tc.tile_pool(name="exps", bufs=1) as expp,
                    tc.tile_pool(name="ysb", bufs=8) as ysbp,
                    tc.tile_pool(name="pad", bufs=4) as padp,
                    tc.tile_pool(name="yT", bufs=3) as yTp,
                ):
                    # DRAM bounce buffers for the three ReduceScatters
                    cc1_i = dramp.tile([1024, 4 * HL * (DH + 1)], bf16, name="cc1_i")
                    cc1_o = dramp.tile([256, 4 * HL * (DH + 1)], bf16, name="cc1_o")
                    cc2_i = dramp.tile([512, 4 * HL * (DH + 1)], bf16, name="cc2_i")
                    cc2_o = dramp.tile([128, 4 * HL * (DH + 1)], bf16, name="cc2_o")
                    cc3_i = dramp.tile([512, 4 * HL * (DH + 1)], bf16, name="cc3_i")
                    cc3_o = dramp.tile([128, 4 * HL * (DH + 1)], bf16, name="cc3_o")

                    # expST tiles allocated per (ci, h) below
                    for ci in range(4):
                        i0 = ci * 512
                        npair = 2 * (ci + 1)
                        exps = {}
                        for h in range(HL):
                            qt, qoff = qslices[h]
                            kt, koff = kslices[h]
                            for jp in range(npair):
                                et = expp.tile([128, 2, 512], f8, tag=f"e{h}_{jp}",
                                               name=f"e{h}_{jp}")
                                exps[(h, jp)] = et
                                for sl2 in range(2):
                                    jb = 2 * jp + sl2
                                    r = jb - 4 * ci
                                    off = 128 * r if r >= 0 else 0
                                    w = 512 - off
                                    pss = psA.tile([128, 512], f32, tag="psA",
                                                   name="pss")
                                    nc.tensor.matmul(
                                        pss[:, 0:w],
                                        qt[qoff:qoff + 64, jb * 128:(jb + 1) * 128],
                                        kt[koff:koff + 64, i0 + off:i0 + 512],
                                        start=True, stop=True,
                                    )
                                    if r >= 0:
                                        nc.vector.tensor_add(
                                            pss[:, 0:128], pss[:, 0:128], negtri[:])
                                        if off:
                                            nc.vector.memset(et[:, sl2, 0:off], 0.0)
                                    nc.scalar.activation(
                                        et[:, sl2, off:512], pss[:, 0:w],
                                        AF.Exp, scale=0.125,
                                    )
                        # P@V per head: fp8 DoubleRow, y in rows 0:64 and the
                        # softmax denominator (ones-weights chain) in rows 64:128
                        # of the same PSUM bank; row 64 = denominator.
                        psys = []
                        dens = []
                        for h in range(HL):
                            psy = psPV.tile([128, 512], f32, tag="psPV", name="psy")
                            den = psDen.tile([64, 512], f32, tag="psDen", name="den")
                            psys.append(psy)
                            dens.append(den)
                            for jp in range(npair):
                                nc.tensor.matmul(
                                    psy[0:DH, :],
                                    V8[jp][:, :, h, :],
                                    exps[(h, jp)][:],
                                    start=(jp == 0), stop=(jp == npair - 1),
                                    perf_mode=DR,
                                )
                                nc.tensor.matmul(
                                    den[:],
                                    ones8[:],
                                    exps[(h, jp)][:],
                                    start=(jp == 0), stop=(jp == npair - 1),
                                    perf_mode=DR,
                                )
                        # transpose yT -> token-major y_sb tiles
                        y_sb = [ysbp.tile([128, HL * (DH + 1)], bf16, tag="ysb",
                                          name=f"ysb{ib}") for ib in range(4)]
                        for h in range(HL):
                            yT_bf = yTp.tile([128, 512], bf16, tag="yT", name="yT")
                            nc.vector.tensor_copy(yT_bf[0:DH, :], psys[h][0:DH, :])
                            nc.vector.tensor_copy(yT_bf[DH:DH + 1, :], dens[h][0:1, :])
                            for ib in range(4):
                                tp = psTr.tile([128, 128], bf16, tag="psTr",
                                               name="tpy")
                                nc.tensor.matmul(
                                    tp[:], yT_bf[:, ib * 128:(ib + 1) * 128],
                                    ident[:], is_transpose=True,
                                    start=True, stop=True,
                                )
                                nc.vector.tensor_copy(
                                    y_sb[ib][:, h * (DH + 1):(h + 1) * (DH + 1)],
                                    tp[:, 0:DH + 1])
                        # pad into 4 receiver slots and ship to the RS buffer
                        W780 = 4 * HL * (DH + 1)
                        for ib in range(4):
                            pad = padp.tile([128, 4, HL * (DH + 1)], bf16, tag="pad",
                                            name="pad")
                            for t in range(4):
                                nc.vector.tensor_scalar_mul(
                                    pad[:, t, :], y_sb[ib][:], msel[:, t:t + 1])
                            if ci < 2:
                                nc.gpsimd.dma_start(
                                    cc1_i[ib * 256 + ci * 128:
                                          ib * 256 + ci * 128 + 128, :],
                                    pad[:].rearrange("p t d -> p (t d)"),
                                )
                            elif ci == 2:
                                nc.gpsimd.dma_start(
                                    cc2_i[ib * 128:(ib + 1) * 128, :],
                                    pad[:].rearrange("p t d -> p (t d)"),
                                )
                            else:
                                nc.gpsimd.dma_start(
                                    cc3_i[ib * 128:(ib + 1) * 128, :],
                                    pad[:].rearrange("p t d -> p (t d)"),
                                )
                        if ci >= 1:
                            cci, cco = [(cc1_i, cc1_o), (cc2_i, cc2_o),
                                        (cc3_i, cc3_o)][ci - 1]
                            nc.gpsimd.collective_compute(
                                "ReduceScatter",
                                mybir.AluOpType.add,
                                replica_groups=[[0, 1, 2, 3], [4, 5, 6, 7]],
                                ins=[cci[:]],
                                outs=[cco[:]],
                            )

            # ===== Phase D/E: readback, residual, LN2, MLP =====
            W65 = DH + 1
            W195 = HL * W65
            with (
                tc.tile_pool(name="yin", bufs=2) as yinp,
                tc.tile_pool(name="x1p", bufs=1) as x1p,
                tc.tile_pool(name="x1nT", bufs=1) as x1nTp,
                tc.tile_pool(name="h1T", bufs=1) as h1Tp,
                tc.tile_pool(name="stat2", bufs=4) as stat2p,
                tc.tile_pool(name="dtmp", bufs=4) as dtmpp,
                tc.tile_pool(name="outp", bufs=2) as outp,
            ):
                x1 = [x1p.tile([128, C], f32, name=f"x1_{k}") for k in range(4)]
                x1nT = [x1nTp.tile([128, 512], bf16, name=f"x1nT{cb}")
                        for cb in range(NC)]
                h1T = [h1Tp.tile([128, 512], bf16, name=f"h1T{nb}")
                       for nb in range(NF)]

                def strip(k, cc_o, row0):
                    """x1[k] = sel(xn)*g1 + b1r + y/denom; LN2 -> x1nT cols."""
                    yall = yinp.tile([128, 4 * W195], bf16, tag="yin", name="yin")
                    nc.sync.dma_start(yall[:], cc_o[row0:row0 + 128, :])
                    yv = yall[:].rearrange("p (q d) -> p q d", d=W65)
                    rec = dtmpp.tile([128, H], f32, tag="rec", name="rec")
                    nc.vector.tensor_copy(rec[:], yv[:, :, DH])
                    nc.vector.reciprocal(rec[:], rec[:])
                    yf = dtmpp.tile([128, C], f32, tag="yf", name="yf")
                    for hh in range(H):
                        nc.vector.tensor_scalar_mul(
                            yf[:, hh * DH:(hh + 1) * DH],
                            yv[:, hh, 0:DH],
                            rec[:, hh:hh + 1],
                        )
                    # select own xn rows via one-hot identity matmul
                    for gg in range(2):
                        psx = psA.tile([128, 512], f32, tag="psA", name="psx")
                        for t in range(4):
                            nc.tensor.matmul(
                                psx[:, 0:384],
                                selid[:, t * 128:(t + 1) * 128],
                                xn_bf[4 * k + t][:, gg * 384:(gg + 1) * 384],
                                start=(t == 0), stop=(t == 3),
                            )
                        gs = slice(gg * 384, (gg + 1) * 384)
                        nc.vector.tensor_tensor(
                            x1[k][:, gs], psx[:, 0:384], g1b[:, gs], ALU.mult)
                        nc.vector.tensor_add(x1[k][:, gs], x1[k][:, gs], b1rb[:, gs])
                        nc.vector.tensor_add(x1[k][:, gs], x1[k][:, gs], yf[:, gs])
                    # LN2
                    st6 = stat2p.tile([128, 2, 6], f32, tag="st6", name="st6b")
                    for gg in range(2):
                        nc.vector.bn_stats(
                            st6[:, gg, :], x1[k][:, gg * 384:(gg + 1) * 384])
                    st2 = stat2p.tile([128, 2], f32, tag="st2", name="st2b")
                    nc.vector.bn_aggr(st2[:], st6[:])
                    std = stat2p.tile([128, 1], f32, tag="std", name="stdb")
                    nc.scalar.activation(std[:], st2[:, 1:2], AF.Sqrt, bias=eps_t[:])
                    rstd = stat2p.tile([128, 1], f32, tag="rstd", name="rstdb")
                    nc.vector.reciprocal(rstd[:], std[:])
                    nmb = stat2p.tile([128, 1], f32, tag="nmb", name="nmbb")
                    nc.vector.tensor_scalar(
                        nmb[:], st2[:, 0:1], rstd[:], -1.0,
                        op0=ALU.mult, op1=ALU.mult,
                    )
                    x1n = dtmpp.tile([128, C], bf16, tag="x1n", name="x1n")
                    nc.scalar.activation(
                        x1n[:], x1[k][:], AF.Identity, bias=nmb[:], scale=rstd[:])
                    for cb in range(NC):
                        tp = psTr.tile([128, 128], bf16, tag="psTr", name="tpb")
                        nc.tensor.matmul(
                            tp[:], x1n[:, cb * 128:(cb + 1) * 128],
                            ident[:], is_transpose=True, start=True, stop=True,
                        )
                        if cb % 2 == 0:
                            nc.scalar.copy(x1nT[cb][:, k * 128:(k + 1) * 128], tp[:])
                        else:
                            nc.vector.tensor_copy(
                                x1nT[cb][:, k * 128:(k + 1) * 128], tp[:])

                def mlp(col0, w):
                    """h1T[:, col0:col0+w] = gelu(W1^T x1nT + b1)."""
                    for nb in range(NF):
                        ps = psA.tile([128, 512], f32, tag="psA", name="psh")
                        for cb in range(NC):
                            nc.tensor.matmul(
                                ps[:, 0:w],
                                w1[cb][:, nb * 128:(nb + 1) * 128],
                                x1nT[cb][:, col0:col0 + w],
                                start=(cb == 0), stop=(cb == NC - 1),
                            )
                        nc.scalar.activation(
                            h1T[nb][:, col0:col0 + w], ps[:, 0:w],
                            AF.Gelu, bias=b1s[:, nb:nb + 1],
                        )

                def outproj(k):
                    o_sb = outp.tile([128, C], f32, tag="o", name="o_sb")
                    for gg in range(2):
                        ps = psA.tile([128, 512], f32, tag="psA", name="pso")
                        for nb in range(NF):
                            nc.tensor.matmul(
                                ps[:, 0:384],
                                h1T[nb][:, k * 128:(k + 1) * 128],
                                w2[nb][:, gg * 384:(gg + 1) * 384],
                                start=(nb == 0), stop=False,
                            )
                        nc.tensor.matmul(
                            ps[:, 0:384], ones_col[:],
                            b2s[:, gg * 384:(gg + 1) * 384],
                            start=False, stop=True,
                        )
                        nc.vector.tensor_add(
                            o_sb[:, gg * 384:(gg + 1) * 384], ps[:, 0:384],
                            x1[k][:, gg * 384:(gg + 1) * 384],
                        )
                    nc.sync.dma_start(out_d[k * 128:(k + 1) * 128, :], o_sb[:])

                strip(0, cc1_o, 0)
                strip(1, cc1_o, 128)
                strip(2, cc2_o, 0)
                mlp(0, 384)
                for k in range(3):
                    outproj(k)
                strip(3, cc3_o, 0)
                mlp(384, 128)
                outproj(3)

            w2p.release()
            w1p.release()

    nc.compile()
    return nc


def _prep_inputs(inputs):
    import ml_dtypes

    f = np.float32
    bf = ml_dtypes.bfloat16
    g1 = np.asarray(inputs["ln1_g"], f)
    b1r = np.asarray(inputs["ln1_b"], f)
    g2 = np.asarray(inputs["ln2_g"], f)
    b2r = np.asarray(inputs["ln2_b"], f)
    Wq, Wk, Wv = (np.asarray(inputs[k], f) for k in ("Wq", "Wk", "Wv"))
    W1, W2 = np.asarray(inputs["W1"], f), np.asarray(inputs["W2"], f)
    x = np.asarray(inputs["x"], f)

    def c(a, dtype=bf):
        return np.ascontiguousarray(a.astype(dtype))

    wq_f = g1[:, None] * Wq
    wk_f = g1[:, None] * Wk
    wv_f = g1[:, None] * Wv
    bq_f = b1r @ Wq + np.asarray(inputs["bq"], f)
    bk_f = b1r @ Wk + np.asarray(inputs["bk"], f)
    bv_f = b1r @ Wv + np.asarray(inputs["bv"], f)
    b1_f = b2r @ W1 + np.asarray(inputs["b1"], f)

    shared = {
        "w1": c(g2[:, None] * W1),
        "b1": np.ascontiguousarray(b1_f.reshape(NF, 128).T).astype(f),
        "w2": c(W2),
        "b2": c(np.asarray(inputs["b2"], f)[None, :]),
        "g1": c(g1[None, :]),
        "b1r": c(b1r[None, :]),
        "negtri": np.ascontiguousarray(
            NEG * np.tril(np.ones((128, 128), f), -1)).astype(f),
        "ident": c(np.eye(128, dtype=f)),
    }

    def bias2(b):
        out = np.zeros((128, 2), f)
        out[:, 0] = b[0:128]
        out[0:64, 1] = b[128:192]
        return out

    in_maps = []
    for core in range(8):
        b, g = core // 4, core % 4
        cols = slice(192 * g, 192 * (g + 1))
        msel = np.zeros((128, 4), f)
        msel[:, g] = 1.0
        selid = np.zeros((128, 512), f)
        selid[:, 128 * g:128 * (g + 1)] = np.eye(128, dtype=f)
        wq_g = wq_f[:, cols]
        wk_g = wk_f[:, cols]
        bq_g = bq_f[cols]
        bk_g = bk_f[cols]
        m = dict(shared)
        m["x"] = c(x[b])
        m["wq"] = c(wq_g[:, 0:128])
        m["wk"] = c(wk_g[:, 0:128])
        m["wqkc"] = c(np.concatenate([wq_g[:, 128:192], wk_g[:, 128:192]], axis=1))
        m["bqkc"] = np.ascontiguousarray(
            np.concatenate([bq_g[128:192], bk_g[128:192]])[:, None]).astype(f)
        m["wv"] = c(wv_f[:, cols])
        m["bq"] = bias2(bq_g)
        m["bk"] = bias2(bk_g)
        m["bv"] = c(bv_f[cols][None, :])
        m["msel"] = msel
        m["selid"] = c(selid)
        in_maps.append(m)
    return in_maps


def kernel(**inputs):
    from concourse.bass_utils import run_bass_kernel_spmd

    if "nc" not in _CACHE:
        _CACHE["nc"] = _build_program()
    nc = _CACHE["nc"]

    in_maps = _prep_inputs(inputs)

    trace = bool(int(os.environ.get("KERNEL_TRACE", "0")))
    try:
        res = run_bass_kernel_spmd(nc, in_maps, core_ids=list(range(8)), trace=trace)
    except ModuleNotFoundError:
        res = run_bass_kernel_spmd(nc, in_maps, core_ids=list(range(8)), trace=False)
    _CACHE["last_result"] = res

    out = np.empty((B, T, C), np.float32)
    for core in range(8):
        b, g = core // 4, core % 4
        r = res.results[core]["out"]
        for k in range(4):
            out[b, 512 * k + 128 * g:512 * k + 128 * g + 128] = \
                r[128 * k:128 * (k + 1)]
    return out
